# revision 8
# baseline (speedup 1.0000x reference)
"""Trainium2 Bass kernel for nn_AutoGenReview_21114059227695 (moe_routing).

Strategy: fully data-parallel over batch B=128 across 8 NeuronCores
(16 samples per core, strided assignment for load balance since lengths
are sorted descending). Each core:
  1. gathers its user/item/word embedding rows on-device (indirect DMA)
  2. runs the domain-routed expert MLP (all 4 experts + one-hot select)
  3. runs the LSTM in feature-major layout (gates.T packed in PSUM)
  4. projects its own packed hidden states onto the full vocab (row-
     parallel projection, bf16 weights, fp32 accumulate)
Host reassembles the time-major packed output rows and adds linear_b.
"""

import sys

for _p in ("/opt/trn_rl_repo",):
    if _p not in sys.path:
        sys.path.insert(0, _p)

import numpy as np
import ml_dtypes

import concourse.bass as bass
import concourse.tile as tile
from concourse import mybir
from concourse.bass_utils import run_bass_kernel_spmd
from concourse.masks import make_identity
from concourse.vector_clock import ScopedClock, VectorClock

F32 = mybir.dt.float32
BF16 = mybir.dt.bfloat16
I32 = mybir.dt.int32
AF = mybir.ActivationFunctionType

# problem constants
B, T = 128, 32
U, D, NI = 200000, 4, 50000
UD, ID = 64, 64
V, VW, H = 32000, 128, 256
LSTM_IN = VW + UD + ID  # 256
NCORES = 8
BC = B // NCORES  # 16 samples per core
PPAD = T * BC  # 512 padded packed columns per core
NV_CHUNK = 512  # vocab tile (psum bank limit, fp32)


# ---------------------------------------------------------------------------
# Tile drain workaround: walrus on this image rejects >1 sem wait on the
# final TPB_CTRL drain; split the waits across a chain of SP nops.
def _patched_drain_and_barrier(self, tick_clock, wait_clock):
    nc = self.nc
    gc = tick_clock.global_clock
    nprocs = len(gc)
    for i in range(nprocs):
        t = gc[i]
        if t > 0:
            nop_inst = nc.sync.nop(nofuse=True, hint=f"drain_split_{i}")
            vc = VectorClock([0] * nprocs)
            vc.require_at_least(i, t)
            wait_clock.add_sem_waits(nop_inst.ins, ScopedClock({None: vc}))
    nc.sync.drain()
    nc.all_engine_barrier()
    assert self.sems is not None
    popped = nc._tile_sem_poison_stack.pop()
    assert popped is self._sem_poison
    nc.clear_and_free_semaphores(list(self.sems.allocated().values()))
    nc.all_engine_barrier()


tile.TileContext._drain_and_barrier = _patched_drain_and_barrier


def _split_multiwaits(nc):
    """walrus on this image encodes at most one sem-wait per instruction;
    hoist extra waits onto single-wait NoOps immediately preceding the
    instruction on the same engine (in-order sequencers make this
    semantically identical)."""
    fn = nc.m.functions[0]
    ctr = 0
    for b in fn.blocks:
        out = []
        changed = False
        for inst in b.instructions:
            si = inst.sync_info
            if si is not None and len(si.on_wait) > 1:
                changed = True
                waits = list(si.on_wait)
                for w in waits[:-1]:
                    ctr += 1
                    out.append(
                        mybir.InstNoOp(
                            name=f"WSPLIT-{ctr}",
                            engine=inst.engine,
                            sync_info=mybir.SyncInfo(on_wait=[w], on_update=[]),
                        )
                    )
                inst.sync_info = mybir.SyncInfo(
                    on_wait=[waits[-1]], on_update=list(si.on_update)
                )
            out.append(inst)
        if changed:
            b.instructions = out


# ---------------------------------------------------------------------------
def build_program():
    nc = bass.Bass()

    # ---- DRAM parameters (identical shapes on every core) ----
    d_uidx = nc.declare_dram_parameter("uidx", [128, 1], I32, isOutput=False)
    d_vidx = nc.declare_dram_parameter("vidx", [128, 1], I32, isOutput=False)
    d_ridx = nc.declare_dram_parameter("ridx", [128, 4], I32, isOutput=False)
    d_utab = nc.declare_dram_parameter("utab", [U, UD], F32, isOutput=False)
    d_itab = nc.declare_dram_parameter("itab", [D * NI, ID], F32, isOutput=False)
    d_wtab = nc.declare_dram_parameter("wtab", [V, VW], F32, isOutput=False)
    d_wih = nc.declare_dram_parameter("wihT", [2, 128, 4 * H], BF16, isOutput=False)
    d_whh = nc.declare_dram_parameter("whhT", [2, 128, 4 * H], BF16, isOutput=False)
    d_bias = nc.declare_dram_parameter("biasg", [128, 8], F32, isOutput=False)
    d_w0 = nc.declare_dram_parameter("w0", [D, 128, 128], F32, isOutput=False)
    d_b0 = nc.declare_dram_parameter("b0t", [128, D], F32, isOutput=False)
    d_w1 = nc.declare_dram_parameter("w1", [D, 128, 64], F32, isOutput=False)
    d_b1 = nc.declare_dram_parameter("b1t", [64, D], F32, isOutput=False)
    d_aw = nc.declare_dram_parameter("affw", [64, 1], F32, isOutput=False)
    d_ab = nc.declare_dram_parameter("affb", [1, 1], F32, isOutput=False)
    d_mask = nc.declare_dram_parameter("dmask", [1, D * BC], F32, isOutput=False)
    d_lin = nc.declare_dram_parameter("linT", [2, 128, V], BF16, isOutput=False)
    d_out = nc.declare_dram_parameter("outp", [PPAD, V], F32, isOutput=True)
    d_rate = nc.declare_dram_parameter("rating", [1, BC], F32, isOutput=True)

    with tile.TileContext(nc) as tc:
        with (
            tc.tile_pool(name="const", bufs=1) as cpool,
            tc.tile_pool(name="work", bufs=2) as wpool,
            tc.tile_pool(name="big", bufs=1) as bigpool,
            tc.tile_pool(name="lin", bufs=3) as linpool,
            tc.tile_pool(name="ob", bufs=3) as obpool,
            tc.tile_pool(name="ps", bufs=2, space="PSUM") as pspool,
            tc.tile_pool(name="psproj", bufs=3, space="PSUM") as psproj,
            tc.tile_pool(name="psg", bufs=2, space="PSUM") as psg_pool,
        ):
            # ---- load constants/weights into SBUF ----
            ident = cpool.tile([128, 128], F32, tag="ident")
            make_identity(nc, ident[:])

            sb_wih = cpool.tile([128, 2, 4 * H], BF16, tag="wih")
            nc.sync.dma_start(out=sb_wih[:], in_=d_wih[:].rearrange("k p n -> p k n"))
            sb_whh = cpool.tile([128, 2, 4 * H], BF16, tag="whh")
            nc.sync.dma_start(out=sb_whh[:], in_=d_whh[:].rearrange("k p n -> p k n"))
            sb_bias = cpool.tile([128, 8], F32, tag="biasg")
            nc.sync.dma_start(out=sb_bias[:], in_=d_bias[:])

            sb_w0 = cpool.tile([128, D, 128], F32, tag="w0")
            nc.sync.dma_start(out=sb_w0[:], in_=d_w0[:].rearrange("d k m -> k d m"))
            sb_b0 = cpool.tile([128, D], F32, tag="b0")
            nc.sync.dma_start(out=sb_b0[:], in_=d_b0[:])
            sb_w1 = cpool.tile([128, D, 64], F32, tag="w1")
            nc.sync.dma_start(out=sb_w1[:], in_=d_w1[:].rearrange("d k m -> k d m"))
            sb_b1 = cpool.tile([64, D], F32, tag="b1")
            nc.sync.dma_start(out=sb_b1[:], in_=d_b1[:])
            sb_aw = cpool.tile([64, 1], F32, tag="affw")
            nc.sync.dma_start(out=sb_aw[:], in_=d_aw[:])
            sb_ab = cpool.tile([1, 1], F32, tag="affb")
            nc.sync.dma_start(out=sb_ab[:], in_=d_ab[:])
            sb_mask = cpool.tile([1, D * BC], F32, tag="dmask")
            nc.sync.dma_start(out=sb_mask[:], in_=d_mask[:])

            sb_uidx = cpool.tile([128, 1], I32, tag="uidx")
            nc.sync.dma_start(out=sb_uidx[:], in_=d_uidx[:])
            sb_vidx = cpool.tile([128, 1], I32, tag="vidx")
            nc.sync.dma_start(out=sb_vidx[:], in_=d_vidx[:])
            sb_ridx = cpool.tile([128, 4], I32, tag="ridx")
            nc.sync.dma_start(out=sb_ridx[:], in_=d_ridx[:])

            # ---- embedding gathers (indirect DMA: one row per partition) ----
            uv_sb = wpool.tile([128, 128], F32, tag="uvrows")
            nc.gpsimd.indirect_dma_start(
                out=uv_sb[:, 0:UD],
                out_offset=None,
                in_=d_utab[:],
                in_offset=bass.IndirectOffsetOnAxis(ap=sb_uidx[:, :1], axis=0),
            )
            nc.gpsimd.indirect_dma_start(
                out=uv_sb[:, UD : UD + ID],
                out_offset=None,
                in_=d_itab[:],
                in_offset=bass.IndirectOffsetOnAxis(ap=sb_vidx[:, :1], axis=0),
            )

            # uvT: [uv-feature 128, sample 16]
            ps_t = pspool.tile([128, 128], F32, tag="scratch")
            nc.tensor.transpose(out=ps_t[:], in_=uv_sb[:], identity=ident[:])
            uvT_f = wpool.tile([128, BC], F32, tag="uvT_f")
            nc.vector.tensor_copy(uvT_f[:], ps_t[:, 0:BC])
            uvT_b = wpool.tile([128, BC], BF16, tag="uvT_b")
            nc.vector.tensor_copy(uvT_b[:], ps_t[:, 0:BC])

            # word embeddings: gather 512 token rows (t-major), transpose to
            # wT [VW=128, 512]
            wT = bigpool.tile([128, T * BC], BF16, tag="wT")
            for g in range(4):
                wg = wpool.tile([128, 128], F32, tag="wg")
                nc.gpsimd.indirect_dma_start(
                    out=wg[:],
                    out_offset=None,
                    in_=d_wtab[:],
                    in_offset=bass.IndirectOffsetOnAxis(ap=sb_ridx[:, g : g + 1], axis=0),
                )
                ps_w = pspool.tile([128, 128], F32, tag="scratch")
                nc.tensor.transpose(out=ps_w[:], in_=wg[:], identity=ident[:])
                nc.vector.tensor_copy(wT[:, g * 128 : (g + 1) * 128], ps_w[:])

            # ---- expert MLP (all 4 domains, one-hot select) ----
            ps_e1 = pspool.tile([128, D * BC], F32, tag="scratch")
            h1 = wpool.tile([128, D * BC], F32, tag="h1")
            for d in range(D):
                nc.tensor.matmul(
                    out=ps_e1[:, d * BC : (d + 1) * BC],
                    lhsT=sb_w0[:, d, :],
                    rhs=uvT_f[:],
                    start=True,
                    stop=True,
                )
                nc.scalar.activation(
                    out=h1[:, d * BC : (d + 1) * BC],
                    in_=ps_e1[:, d * BC : (d + 1) * BC],
                    func=AF.Relu,
                    bias=sb_b0[:, d : d + 1],
                )
            ps_e2 = pspool.tile([64, D * BC], F32, tag="scratch")
            h2 = wpool.tile([64, D * BC], F32, tag="h2")
            for d in range(D):
                nc.tensor.matmul(
                    out=ps_e2[:, d * BC : (d + 1) * BC],
                    lhsT=sb_w1[:, d, :],
                    rhs=h1[:, d * BC : (d + 1) * BC],
                    start=True,
                    stop=True,
                )
                nc.scalar.activation(
                    out=h2[:, d * BC : (d + 1) * BC],
                    in_=ps_e2[:, d * BC : (d + 1) * BC],
                    func=AF.Relu,
                    bias=sb_b1[:, d : d + 1],
                )
            ps_e3 = pspool.tile([1, D * BC], F32, tag="scratch")
            for d in range(D):
                nc.tensor.matmul(
                    out=ps_e3[:, d * BC : (d + 1) * BC],
                    lhsT=sb_aw[:],
                    rhs=h2[:, d * BC : (d + 1) * BC],
                    start=True,
                    stop=True,
                )
            r_sig = wpool.tile([1, D * BC], F32, tag="r_sig")
            nc.scalar.activation(out=r_sig[:], in_=ps_e3[:], func=AF.Sigmoid, bias=sb_ab[0:1, 0:1])
            r5 = wpool.tile([1, D * BC], F32, tag="r5")
            nc.vector.tensor_scalar_mul(r5[:], r_sig[:], 5.0)
            nc.vector.tensor_mul(r5[:], r5[:], sb_mask[:])
            r_fin = wpool.tile([1, BC], F32, tag="r_fin")
            nc.vector.tensor_add(r_fin[:], r5[:, 0:BC], r5[:, BC : 2 * BC])
            nc.vector.tensor_add(r_fin[:], r_fin[:], r5[:, 2 * BC : 3 * BC])
            nc.vector.tensor_add(r_fin[:], r_fin[:], r5[:, 3 * BC : 4 * BC])
            nc.sync.dma_start(out=d_rate[:], in_=r_fin[:])

            # ---- LSTM bulk x-part ----
            # SBIAS[p, m*16+j] = (w_ih[:, :128] @ uv.T)[m*128+p, j] + (b_ih+b_hh)[m*128+p]
            ps_guv = psg_pool.tile([128, 128], F32, tag="gate")
            for m in range(8):
                nc.tensor.matmul(
                    out=ps_guv[:, m * BC : (m + 1) * BC],
                    lhsT=sb_wih[:, 0, m * 128 : (m + 1) * 128],
                    rhs=uvT_b[:],
                    start=True,
                    stop=True,
                )
            sbias = wpool.tile([128, 128], F32, tag="sbias")
            for m in range(8):
                nc.scalar.activation(
                    out=sbias[:, m * BC : (m + 1) * BC],
                    in_=ps_guv[:, m * BC : (m + 1) * BC],
                    func=AF.Identity,
                    bias=sb_bias[:, m : m + 1],
                )

            # gxp[p, t*128 + m*16 + j] = (w_ih[:,128:] @ wemb.T)[m*128+p, (t,j)] + SBIAS
            gxp = bigpool.tile([128, T * 128], F32, tag="gxp")
            gxp_v = gxp[:].rearrange("p (t m j) -> p m j t", t=T, m=8)
            for m in range(8):
                ps_b = pspool.tile([128, 512], F32, tag="scratch")
                nc.tensor.matmul(
                    out=ps_b[:],
                    lhsT=sb_wih[:, 1, m * 128 : (m + 1) * 128],
                    rhs=wT[:],
                    start=True,
                    stop=True,
                )
                nc.vector.tensor_add(
                    out=gxp_v[:, m],
                    in0=ps_b[:].rearrange("p (t j) -> p j t", t=T),
                    in1=sbias[:, m * BC : (m + 1) * BC].to_broadcast([128, BC, T]),
                )

            # ---- LSTM recurrence (feature-major, packed gates) ----
            # hsP[p, k*512 + t*16 + j] = h_t[k*128+p, j]   (bf16)
            # (k-planes keep projection lhsT chunks single-free-dim contiguous)
            hsP = bigpool.tile([128, 2 * T * BC], BF16, tag="hsP")
            hsP_v = hsP[:].rearrange("p (k t j) -> p t k j", k=2, t=T)
            hsP_w = hsP[:].rearrange("p (k t j) -> p t k j", k=2, t=T)
            c_bufs = [
                bigpool.tile([128, 2 * BC], F32, name=f"cbuf{i}", tag=f"cbuf{i}")
                for i in range(2)
            ]
            A = None
            for t in range(T):
                if t == 0:
                    gsrc = gxp[:, 0:128]
                else:
                    ps_g = psg_pool.tile([128, 128], F32, tag="gate")
                    for m in range(8):
                        for k in range(2):
                            nc.tensor.matmul(
                                out=ps_g[:, m * BC : (m + 1) * BC],
                                lhsT=sb_whh[:, k, m * 128 : (m + 1) * 128],
                                rhs=hsP_v[:, t - 1, k],
                                start=(k == 0),
                                stop=(k == 1),
                            )
                    G = wpool.tile([128, 128], F32, tag="Gt")
                    nc.vector.tensor_add(G[:], ps_g[:], gxp[:, t * 128 : (t + 1) * 128])
                    gsrc = G[:]
                A = wpool.tile([128, 128], F32, tag="At")
                nc.scalar.activation(out=A[:, 0:64], in_=gsrc[:, 0:64], func=AF.Sigmoid)
                nc.scalar.activation(out=A[:, 64:96], in_=gsrc[:, 64:96], func=AF.Tanh)
                nc.scalar.activation(out=A[:, 96:128], in_=gsrc[:, 96:128], func=AF.Sigmoid)
                t1 = wpool.tile([128, 2 * BC], F32, tag="t1")
                nc.vector.tensor_mul(t1[:], A[:, 0:32], A[:, 64:96])
                c_new = c_bufs[t % 2]
                if t == 0:
                    nc.vector.tensor_copy(c_new[:], t1[:])
                else:
                    c_old = c_bufs[(t - 1) % 2]
                    nc.vector.tensor_mul(c_new[:], A[:, 32:64], c_old[:])
                    nc.vector.tensor_add(c_new[:], c_new[:], t1[:])
                tct = wpool.tile([128, 2 * BC], F32, tag="tct")
                nc.scalar.activation(out=tct[:], in_=c_new[:], func=AF.Tanh)
                nc.vector.tensor_mul(
                    out=hsP_w[:, t],
                    in0=A[:, 96:128].rearrange("p (k j) -> p k j", k=2),
                    in1=tct[:].rearrange("p (k j) -> p k j", k=2),
                )

            # ---- vocab projection: out[tj, v] = hs[tj, :] @ linear_w.T ----
            hsP_k = hsP[:].rearrange("p (k m) -> p k m", k=2)
            nfull, nrem = divmod(V, NV_CHUNK)
            nchunks = nfull + (1 if nrem else 0)
            for n in range(nchunks):
                nn_ = NV_CHUNK if n < nfull else nrem
                nsl = slice(n * NV_CHUNK, n * NV_CHUNK + nn_)
                lin_sb = linpool.tile([128, 2, NV_CHUNK], BF16, tag="lin")
                nc.sync.dma_start(
                    out=lin_sb[:, :, 0:nn_],
                    in_=d_lin[:].rearrange("k p n -> p k n")[:, :, nsl],
                )
                for mc in range(PPAD // 128):
                    ps_p = psproj.tile([128, NV_CHUNK], F32, tag="ps_proj")
                    for k in range(2):
                        nc.tensor.matmul(
                            out=ps_p[:, 0:nn_],
                            lhsT=hsP_k[:, k, mc * 128 : (mc + 1) * 128],
                            rhs=lin_sb[:, k, 0:nn_],
                            start=(k == 0),
                            stop=(k == 1),
                        )
                    ob = obpool.tile([128, NV_CHUNK], F32, tag="ob")
                    nc.vector.tensor_copy(ob[:, 0:nn_], ps_p[:, 0:nn_])
                    nc.gpsimd.dma_start(
                        out=d_out[mc * 128 : (mc + 1) * 128, nsl], in_=ob[:, 0:nn_]
                    )
    _split_multiwaits(nc)
    return nc


_NC_CACHE = None


def _get_nc():
    global _NC_CACHE
    if _NC_CACHE is None:
        _NC_CACHE = build_program()
    return _NC_CACHE


def _prep_inputs(inputs):
    """Build the 8 per-core in_maps + host-side assembly metadata."""
    u_idx = np.asarray(inputs["u_idx"]).astype(np.int32)
    i_idx = np.asarray(inputs["i_idx"]).astype(np.int32)
    d_idx = np.asarray(inputs["d_idx"]).astype(np.int32)
    review = np.asarray(inputs["review"]).astype(np.int32)
    length = np.asarray(inputs["length"]).astype(np.int32)

    utab = np.ascontiguousarray(np.asarray(inputs["emb_users_w"], np.float32))
    itab = np.ascontiguousarray(
        np.asarray(inputs["emb_items_w"], np.float32).reshape(D * NI, ID)
    )
    wtab = np.ascontiguousarray(np.asarray(inputs["word_emb_w"], np.float32))

    w_ih = np.asarray(inputs["w_ih"], np.float32)
    w_hh = np.asarray(inputs["w_hh"], np.float32)
    bsum = (np.asarray(inputs["b_ih"], np.float32) + np.asarray(inputs["b_hh"], np.float32))
    wihT = np.ascontiguousarray(
        w_ih.T.reshape(2, 128, 4 * H).astype(ml_dtypes.bfloat16)
    )
    whhT = np.ascontiguousarray(
        w_hh.T.reshape(2, 128, 4 * H).astype(ml_dtypes.bfloat16)
    )
    biasg = np.ascontiguousarray(bsum.reshape(8, 128).T)

    w0 = np.ascontiguousarray(np.asarray(inputs["fc_w0"], np.float32))
    b0t = np.ascontiguousarray(np.asarray(inputs["fc_b0"], np.float32).T)
    w1 = np.ascontiguousarray(np.asarray(inputs["fc_w1"], np.float32))
    b1t = np.ascontiguousarray(np.asarray(inputs["fc_b1"], np.float32).T)
    affw = np.ascontiguousarray(np.asarray(inputs["affine_w"], np.float32))
    affb = np.asarray(inputs["affine_b"], np.float32).reshape(1, 1)

    linear_w = np.asarray(inputs["linear_w"], np.float32)
    linT = np.ascontiguousarray(
        linear_w.T.reshape(2, 128, V).astype(ml_dtypes.bfloat16)
    )

    in_maps = []
    meta = []
    for c in range(NCORES):
        samples = c + NCORES * np.arange(BC)  # descending lengths
        uidx_p = np.zeros((128, 1), np.int32)
        uidx_p[:BC, 0] = u_idx[samples]
        vidx_p = np.zeros((128, 1), np.int32)
        vidx_p[:BC, 0] = d_idx[samples] * NI + i_idx[samples]
        toks_tm = review[samples].T.reshape(T * BC)  # t-major
        ridx = np.ascontiguousarray(toks_tm.reshape(4, 128).T.astype(np.int32))
        dmask = np.zeros((1, D * BC), np.float32)
        for j, b in enumerate(samples):
            dmask[0, d_idx[b] * BC + j] = 1.0
        in_maps.append(
            {
                "uidx": uidx_p,
                "vidx": vidx_p,
                "ridx": ridx,
                "utab": utab,
                "itab": itab,
                "wtab": wtab,
                "wihT": wihT,
                "whhT": whhT,
                "biasg": biasg,
                "w0": w0,
                "b0t": b0t,
                "w1": w1,
                "b1t": b1t,
                "affw": affw,
                "affb": affb,
                "dmask": dmask,
                "linT": linT,
            }
        )
        meta.append((samples, length[samples]))
    return in_maps, meta, length, np.asarray(inputs["linear_b"], np.float32)


def _assemble(results, meta, length, linear_b):
    counts = (length[None, :] > np.arange(T)[:, None]).sum(1)  # per-t valid count
    cum = np.concatenate([[0], np.cumsum(counts)])
    P = int(cum[-1])
    outputs = np.empty((P, V), np.float32)
    rating = np.empty((B, 1), np.float32)
    for c in range(NCORES):
        samples, lens = meta[c]
        res = results[c]
        rating[samples, 0] = res["rating"][0]
        tt, jj = np.meshgrid(np.arange(T), np.arange(BC), indexing="ij")
        valid = tt < lens[jj]
        src = (tt * BC + jj)[valid]
        dst = (cum[tt] + samples[jj])[valid]
        outputs[dst] = res["outp"][src]
    outputs += linear_b[None, :]
    return rating, outputs


def run(inputs, trace=False):
    nc = _get_nc()
    in_maps, meta, length, linear_b = _prep_inputs(inputs)
    res = run_bass_kernel_spmd(
        nc, in_maps, core_ids=list(range(NCORES)), trace=trace
    )
    rating, outputs = _assemble(res.results, meta, length, linear_b)
    return rating, outputs, res


def kernel(**inputs):
    rating, outputs, _ = run(inputs, trace=False)
    return rating, outputs


# revision 12
# speedup vs baseline: 1.2182x; 1.2182x over previous
"""Trainium2 Bass kernel for nn_AutoGenReview_21114059227695 (moe_routing).

Strategy: fully data-parallel over batch B=128 across 8 NeuronCores
(16 samples per core, strided assignment for load balance since lengths
are sorted descending). Each core:
  1. gathers its user/item/word embedding rows on-device (indirect DMA)
  2. runs the domain-routed expert MLP (all 4 experts + one-hot select)
  3. runs the LSTM in feature-major layout (gates.T packed in PSUM; the
     x-part of the gates is bulk-precomputed and re-injected into PSUM
     through an identity matmul so ScalarE reads gates straight from PSUM)
  4. projects its packed hidden states onto the full vocab (row-parallel
     projection, bf16 weights resident in SBUF, fp32 accumulate),
     interleaved into the LSTM step loop as packed columns become ready
Host reassembles the time-major packed output rows and adds linear_b.
"""

import sys

for _p in ("/opt/trn_rl_repo",):
    if _p not in sys.path:
        sys.path.insert(0, _p)

import numpy as np
import ml_dtypes

import concourse.bass as bass
import concourse.tile as tile
from concourse import mybir
from concourse.bass_utils import run_bass_kernel_spmd
from concourse.masks import make_identity
from concourse.vector_clock import ScopedClock, VectorClock

F32 = mybir.dt.float32
BF16 = mybir.dt.bfloat16
I32 = mybir.dt.int32
AF = mybir.ActivationFunctionType

# problem constants
B, T = 128, 32
U, D, NI = 200000, 4, 50000
UD, ID = 64, 64
V, VW, H = 32000, 128, 256
NCORES = 8
BC = B // NCORES  # 16 samples per core
NV_CHUNK = 512  # vocab tile (psum bank limit, fp32)

# gate permutation: torch order [i f g o] -> kernel order [i f o g] so a
# single ScalarE sigmoid covers cols 0:96 and tanh covers 96:128
GATE_PERM = np.r_[0:256, 256:512, 768:1024, 512:768]


# ---------------------------------------------------------------------------
# Tile drain workaround: walrus on this image rejects >1 sem wait on the
# final TPB_CTRL drain; split the waits across a chain of SP nops.
def _patched_drain_and_barrier(self, tick_clock, wait_clock):
    nc = self.nc
    gc = tick_clock.global_clock
    nprocs = len(gc)
    for i in range(nprocs):
        t = gc[i]
        if t > 0:
            nop_inst = nc.sync.nop(nofuse=True, hint=f"drain_split_{i}")
            vc = VectorClock([0] * nprocs)
            vc.require_at_least(i, t)
            wait_clock.add_sem_waits(nop_inst.ins, ScopedClock({None: vc}))
    nc.sync.drain()
    nc.all_engine_barrier()
    assert self.sems is not None
    popped = nc._tile_sem_poison_stack.pop()
    assert popped is self._sem_poison
    nc.clear_and_free_semaphores(list(self.sems.allocated().values()))
    nc.all_engine_barrier()


tile.TileContext._drain_and_barrier = _patched_drain_and_barrier


def _split_multiwaits(nc):
    """walrus on this image encodes at most one sem-wait per instruction;
    hoist extra waits onto single-wait NoOps immediately preceding the
    instruction on the same engine (in-order sequencers make this
    semantically identical)."""
    fn = nc.m.functions[0]
    ctr = 0
    for b in fn.blocks:
        out = []
        changed = False
        for inst in b.instructions:
            si = inst.sync_info
            if si is not None and len(si.on_wait) > 1:
                changed = True
                waits = list(si.on_wait)
                for w in waits[:-1]:
                    ctr += 1
                    out.append(
                        mybir.InstNoOp(
                            name=f"WSPLIT-{ctr}",
                            engine=inst.engine,
                            sync_info=mybir.SyncInfo(on_wait=[w], on_update=[]),
                        )
                    )
                inst.sync_info = mybir.SyncInfo(
                    on_wait=[waits[-1]], on_update=list(si.on_update)
                )
            out.append(inst)
        if changed:
            b.instructions = out


# ---------------------------------------------------------------------------
def build_program(cm):
    """cm: per-timestep packed column width (max over cores of per-core
    valid-sample counts), non-increasing, cm[0] == BC."""
    cm = list(cm)
    pos = np.concatenate([[0], np.cumsum(cm)]).astype(int)  # packed offsets
    PP = int(pos[-1])  # packed columns per core
    nmc = (PP + 127) // 128  # projection row chunks

    nc = bass.Bass()

    d_uidx = nc.declare_dram_parameter("uidx", [128, 1], I32, isOutput=False)
    d_vidx = nc.declare_dram_parameter("vidx", [128, 1], I32, isOutput=False)
    d_ridx = nc.declare_dram_parameter("ridx", [128, 4], I32, isOutput=False)
    d_utab = nc.declare_dram_parameter("utab", [U, UD], F32, isOutput=False)
    d_itab = nc.declare_dram_parameter("itab", [D * NI, ID], F32, isOutput=False)
    d_wtab = nc.declare_dram_parameter("wtab", [V, VW], F32, isOutput=False)
    d_wih = nc.declare_dram_parameter("wihT", [2, 128, 4 * H], BF16, isOutput=False)
    d_whh = nc.declare_dram_parameter("whhT", [2, 128, 4 * H], BF16, isOutput=False)
    d_bias = nc.declare_dram_parameter("biasg", [128, 8], F32, isOutput=False)
    d_w0 = nc.declare_dram_parameter("w0", [D, 128, 128], BF16, isOutput=False)
    d_b0 = nc.declare_dram_parameter("b0t", [128, D], F32, isOutput=False)
    d_w1 = nc.declare_dram_parameter("w1", [D, 128, 64], BF16, isOutput=False)
    d_b1 = nc.declare_dram_parameter("b1t", [64, D], F32, isOutput=False)
    d_aw = nc.declare_dram_parameter("affw", [64, 1], BF16, isOutput=False)
    d_ab = nc.declare_dram_parameter("affb", [1, 1], F32, isOutput=False)
    d_mask = nc.declare_dram_parameter("dmask", [1, D * BC], F32, isOutput=False)
    d_lin = nc.declare_dram_parameter("linT", [2, 128, V], BF16, isOutput=False)
    d_out = nc.declare_dram_parameter("outp", [PP, V], F32, isOutput=True)
    d_rate = nc.declare_dram_parameter("rating", [1, BC], F32, isOutput=True)

    nfull, nrem = divmod(V, NV_CHUNK)
    nchunks = nfull + (1 if nrem else 0)

    with tile.TileContext(nc) as tc:
        with (
            tc.tile_pool(name="const", bufs=1) as cpool,
            tc.tile_pool(name="work", bufs=2) as wpool,
            tc.tile_pool(name="big", bufs=1) as bigpool,
            tc.tile_pool(name="ob", bufs=4) as obpool,
            tc.tile_pool(name="ps", bufs=2, space="PSUM") as pspool,
            tc.tile_pool(name="psproj", bufs=3, space="PSUM") as psproj,
            tc.tile_pool(name="psg", bufs=2, space="PSUM") as psg_pool,
        ):
            # ---- resident linear_w.T (bf16, 125KB/partition) ----
            lin_sb = cpool.tile([128, 2, V], BF16, tag="lin")
            lin_src = d_lin[:].rearrange("k p n -> p k n")
            dma_engines = [nc.sync, nc.gpsimd, nc.scalar]
            for s in range(8):
                sl = slice(s * (V // 8), (s + 1) * (V // 8))
                dma_engines[s % 3].dma_start(
                    out=lin_sb[:, :, sl], in_=lin_src[:, :, sl]
                )

            # ---- constants/weights ----
            ident = cpool.tile([128, 128], F32, tag="ident")
            make_identity(nc, ident[:])
            ident_b = cpool.tile([128, 128], BF16, tag="identb")
            nc.vector.tensor_copy(ident_b[:], ident[:])

            sb_wih = cpool.tile([128, 2, 4 * H], BF16, tag="wih")
            nc.sync.dma_start(out=sb_wih[:], in_=d_wih[:].rearrange("k p n -> p k n"))
            sb_whh = cpool.tile([128, 2, 4 * H], BF16, tag="whh")
            nc.sync.dma_start(out=sb_whh[:], in_=d_whh[:].rearrange("k p n -> p k n"))
            sb_bias = cpool.tile([128, 8], F32, tag="biasg")
            nc.sync.dma_start(out=sb_bias[:], in_=d_bias[:])

            sb_w0 = cpool.tile([128, D, 128], BF16, tag="w0")
            nc.sync.dma_start(out=sb_w0[:], in_=d_w0[:].rearrange("d k m -> k d m"))
            sb_b0 = cpool.tile([128, D], F32, tag="b0")
            nc.sync.dma_start(out=sb_b0[:], in_=d_b0[:])
            sb_w1 = cpool.tile([128, D, 64], BF16, tag="w1")
            nc.sync.dma_start(out=sb_w1[:], in_=d_w1[:].rearrange("d k m -> k d m"))
            sb_b1 = cpool.tile([64, D], F32, tag="b1")
            nc.sync.dma_start(out=sb_b1[:], in_=d_b1[:])
            sb_aw = cpool.tile([64, 1], BF16, tag="affw")
            nc.sync.dma_start(out=sb_aw[:], in_=d_aw[:])
            sb_ab = cpool.tile([1, 1], F32, tag="affb")
            nc.sync.dma_start(out=sb_ab[:], in_=d_ab[:])
            sb_mask = cpool.tile([1, D * BC], F32, tag="dmask")
            nc.sync.dma_start(out=sb_mask[:], in_=d_mask[:])

            sb_uidx = cpool.tile([128, 1], I32, tag="uidx")
            nc.sync.dma_start(out=sb_uidx[:], in_=d_uidx[:])
            sb_vidx = cpool.tile([128, 1], I32, tag="vidx")
            nc.sync.dma_start(out=sb_vidx[:], in_=d_vidx[:])
            sb_ridx = cpool.tile([128, 4], I32, tag="ridx")
            nc.sync.dma_start(out=sb_ridx[:], in_=d_ridx[:])

            # ---- embedding gathers (indirect DMA: one row per partition) ----
            uv_sb = wpool.tile([128, 128], F32, tag="uvrows")
            nc.gpsimd.indirect_dma_start(
                out=uv_sb[:, 0:UD],
                out_offset=None,
                in_=d_utab[:],
                in_offset=bass.IndirectOffsetOnAxis(ap=sb_uidx[:, :1], axis=0),
            )
            nc.gpsimd.indirect_dma_start(
                out=uv_sb[:, UD : UD + ID],
                out_offset=None,
                in_=d_itab[:],
                in_offset=bass.IndirectOffsetOnAxis(ap=sb_vidx[:, :1], axis=0),
            )

            ps_t = pspool.tile([128, 128], F32, tag="scratch")
            nc.tensor.transpose(out=ps_t[:], in_=uv_sb[:], identity=ident[:])
            uvT_b = wpool.tile([128, BC], BF16, tag="uvT_b")
            nc.vector.tensor_copy(uvT_b[:], ps_t[:, 0:BC])

            wT = bigpool.tile([128, T * BC], BF16, tag="wT")
            for g in range(4):
                wg = wpool.tile([128, 128], F32, tag="wg")
                nc.gpsimd.indirect_dma_start(
                    out=wg[:],
                    out_offset=None,
                    in_=d_wtab[:],
                    in_offset=bass.IndirectOffsetOnAxis(ap=sb_ridx[:, g : g + 1], axis=0),
                )
                ps_w = pspool.tile([128, 128], F32, tag="scratch")
                nc.tensor.transpose(out=ps_w[:], in_=wg[:], identity=ident[:])
                nc.vector.tensor_copy(wT[:, g * 128 : (g + 1) * 128], ps_w[:])

            # ---- expert MLP (all 4 domains, one-hot select) ----
            ps_e1 = pspool.tile([128, D * BC], F32, tag="scratch")
            h1 = wpool.tile([128, D * BC], BF16, tag="h1")
            for d in range(D):
                nc.tensor.matmul(
                    out=ps_e1[:, d * BC : (d + 1) * BC],
                    lhsT=sb_w0[:, d, :],
                    rhs=uvT_b[:],
                    start=True,
                    stop=True,
                )
            for d in range(D):
                nc.scalar.activation(
                    out=h1[:, d * BC : (d + 1) * BC],
                    in_=ps_e1[:, d * BC : (d + 1) * BC],
                    func=AF.Relu,
                    bias=sb_b0[:, d : d + 1],
                )
            ps_e2 = pspool.tile([64, D * BC], F32, tag="scratch")
            h2 = wpool.tile([64, D * BC], BF16, tag="h2")
            for d in range(D):
                nc.tensor.matmul(
                    out=ps_e2[:, d * BC : (d + 1) * BC],
                    lhsT=sb_w1[:, d, :],
                    rhs=h1[:, d * BC : (d + 1) * BC],
                    start=True,
                    stop=True,
                )
            for d in range(D):
                nc.scalar.activation(
                    out=h2[:, d * BC : (d + 1) * BC],
                    in_=ps_e2[:, d * BC : (d + 1) * BC],
                    func=AF.Relu,
                    bias=sb_b1[:, d : d + 1],
                )
            ps_e3 = pspool.tile([1, D * BC], F32, tag="scratch")
            for d in range(D):
                nc.tensor.matmul(
                    out=ps_e3[:, d * BC : (d + 1) * BC],
                    lhsT=sb_aw[:],
                    rhs=h2[:, d * BC : (d + 1) * BC],
                    start=True,
                    stop=True,
                )
            r_sig = wpool.tile([1, D * BC], F32, tag="r_sig")
            nc.scalar.activation(out=r_sig[:], in_=ps_e3[:], func=AF.Sigmoid, bias=sb_ab[0:1, 0:1])
            r5 = wpool.tile([1, D * BC], F32, tag="r5")
            nc.vector.tensor_scalar_mul(r5[:], r_sig[:], 5.0)
            nc.vector.tensor_mul(r5[:], r5[:], sb_mask[:])
            r_fin = wpool.tile([1, BC], F32, tag="r_fin")
            nc.vector.tensor_add(r_fin[:], r5[:, 0:BC], r5[:, BC : 2 * BC])
            nc.vector.tensor_add(r_fin[:], r_fin[:], r5[:, 2 * BC : 3 * BC])
            nc.vector.tensor_add(r_fin[:], r_fin[:], r5[:, 3 * BC : 4 * BC])
            nc.sync.dma_start(out=d_rate[:], in_=r_fin[:])

            # ---- LSTM bulk x-part (bf16 gxp, packed gate layout) ----
            ps_guv = psg_pool.tile([128, 128], F32, tag="gate")
            for m in range(8):
                nc.tensor.matmul(
                    out=ps_guv[:, m * BC : (m + 1) * BC],
                    lhsT=sb_wih[:, 0, m * 128 : (m + 1) * 128],
                    rhs=uvT_b[:],
                    start=True,
                    stop=True,
                )
            sbias = wpool.tile([128, 128], F32, tag="sbias")
            for m in range(8):
                nc.scalar.activation(
                    out=sbias[:, m * BC : (m + 1) * BC],
                    in_=ps_guv[:, m * BC : (m + 1) * BC],
                    func=AF.Identity,
                    bias=sb_bias[:, m : m + 1],
                )
            gxp = bigpool.tile([128, T * 128], BF16, tag="gxp")
            gxp_v = gxp[:].rearrange("p (t m j) -> p m j t", t=T, m=8)
            for m in range(8):
                ps_b = pspool.tile([128, 512], F32, tag="scratch")
                nc.tensor.matmul(
                    out=ps_b[:],
                    lhsT=sb_wih[:, 1, m * 128 : (m + 1) * 128],
                    rhs=wT[:],
                    start=True,
                    stop=True,
                )
                nc.vector.tensor_add(
                    out=gxp_v[:, m],
                    in0=ps_b[:].rearrange("p (t j) -> p j t", t=T),
                    in1=sbias[:, m * BC : (m + 1) * BC].to_broadcast([128, BC, T]),
                )

            # ---- LSTM recurrence + interleaved projection ----
            # hsP[p, k*PP + pos[t] + j] = h_t[k*128+p, j]  (bf16, packed cols)
            hsP = bigpool.tile([128, 2 * PP], BF16, tag="hsP")
            hsP_k = hsP[:].rearrange("p (k m) -> p k m", k=2)
            c_bufs = [
                bigpool.tile([128, 2 * BC], F32, name=f"cbuf{i}", tag=f"cbuf{i}")
                for i in range(2)
            ]

            # projection work queue: tasks appear as packed chunks complete
            tasks = []  # (mc, n)
            emitted_chunks = [False] * nmc
            ob_engines = [nc.sync, nc.gpsimd]
            ob_ctr = [0]

            def emit_proj(ntasks):
                for _ in range(ntasks):
                    if not tasks:
                        return
                    mc, n = tasks.pop(0)
                    mrows = min(128, PP - mc * 128)
                    nn_ = NV_CHUNK if n < nfull else nrem
                    nsl = slice(n * NV_CHUNK, n * NV_CHUNK + nn_)
                    ps_p = psproj.tile([128, NV_CHUNK], F32, tag="proj")
                    for k in range(2):
                        nc.tensor.matmul(
                            out=ps_p[0:mrows, 0:nn_],
                            lhsT=hsP_k[:, k, mc * 128 : mc * 128 + mrows],
                            rhs=lin_sb[:, k, nsl],
                            start=(k == 0),
                            stop=(k == 1),
                        )
                    ob = obpool.tile([128, NV_CHUNK], F32, tag="ob")
                    if ob_ctr[0] % 3 == 2:
                        nc.scalar.copy(ob[0:mrows, 0:nn_], ps_p[0:mrows, 0:nn_])
                    else:
                        nc.vector.tensor_copy(ob[0:mrows, 0:nn_], ps_p[0:mrows, 0:nn_])
                    eng = ob_engines[ob_ctr[0] % 2]
                    ob_ctr[0] += 1
                    eng.dma_start(
                        out=d_out[mc * 128 : mc * 128 + mrows, nsl],
                        in_=ob[0:mrows, 0:nn_],
                    )

            for t in range(T):
                w = cm[t]
                ps_g = psg_pool.tile([128, 128], F32, tag="gate")
                # inject precomputed x-part gates into PSUM via identity matmul
                nc.tensor.matmul(
                    out=ps_g[:],
                    lhsT=ident_b[:],
                    rhs=gxp[:, t * 128 : (t + 1) * 128],
                    start=True,
                    stop=(t == 0),
                    skip_group_check=True,
                )
                if t > 0:
                    for m in range(8):
                        for k in range(2):
                            nc.tensor.matmul(
                                out=ps_g[:, m * BC : m * BC + w],
                                lhsT=sb_whh[:, k, m * 128 : (m + 1) * 128],
                                rhs=hsP_k[:, k, pos[t - 1] : pos[t - 1] + w],
                                start=False,
                                stop=(m == 7 and k == 1),
                                skip_group_check=True,
                            )
                A = wpool.tile([128, 128], F32, tag="At")
                nc.scalar.activation(out=A[:, 0:96], in_=ps_g[:, 0:96], func=AF.Sigmoid)
                nc.scalar.activation(out=A[:, 96:128], in_=ps_g[:, 96:128], func=AF.Tanh)
                t1 = wpool.tile([128, 2 * BC], F32, tag="t1")
                nc.vector.tensor_mul(t1[:], A[:, 0:32], A[:, 96:128])
                c_new = c_bufs[t % 2]
                if t == 0:
                    nc.vector.tensor_copy(c_new[:], t1[:])
                else:
                    c_old = c_bufs[(t - 1) % 2]
                    nc.vector.tensor_mul(c_new[:], A[:, 32:64], c_old[:])
                    nc.vector.tensor_add(c_new[:], c_new[:], t1[:])
                tct = wpool.tile([128, 2 * BC], F32, tag="tct")
                nc.scalar.activation(out=tct[:], in_=c_new[:], func=AF.Tanh)
                # write h packed (only the first cm[t] columns are kept)
                hw = hsP[:].rearrange("p (k m) -> p k m", k=2)[
                    :, :, pos[t] : pos[t] + w
                ]
                nc.vector.tensor_mul(
                    out=hw,
                    in0=A[:, 64:96].rearrange("p (k j) -> p k j", k=2)[:, :, 0:w],
                    in1=tct[:].rearrange("p (k j) -> p k j", k=2)[:, :, 0:w],
                )
                # release projection tasks for chunks fully written
                for mc in range(nmc):
                    if not emitted_chunks[mc] and pos[t + 1] >= min((mc + 1) * 128, PP):
                        emitted_chunks[mc] = True
                        tasks.extend((mc, n) for n in range(nchunks))
                if t >= 1:
                    budget = max(0, int(np.ceil(len(tasks) / max(1, T - 1 - t))))
                    emit_proj(min(budget, 8))
            emit_proj(len(tasks) + 1)

    _split_multiwaits(nc)
    return nc


_NC_CACHE = {}


def _get_nc(cm):
    key = tuple(cm)
    if key not in _NC_CACHE:
        _NC_CACHE[key] = build_program(cm)
    return _NC_CACHE[key]


def _prep_inputs(inputs):
    """Build the 8 per-core in_maps + host-side assembly metadata."""
    u_idx = np.asarray(inputs["u_idx"]).astype(np.int32)
    i_idx = np.asarray(inputs["i_idx"]).astype(np.int32)
    d_idx = np.asarray(inputs["d_idx"]).astype(np.int32)
    review = np.asarray(inputs["review"]).astype(np.int32)
    length = np.asarray(inputs["length"]).astype(np.int32)

    utab = np.ascontiguousarray(np.asarray(inputs["emb_users_w"], np.float32))
    itab = np.ascontiguousarray(
        np.asarray(inputs["emb_items_w"], np.float32).reshape(D * NI, ID)
    )
    wtab = np.ascontiguousarray(np.asarray(inputs["word_emb_w"], np.float32))

    w_ih = np.asarray(inputs["w_ih"], np.float32)[GATE_PERM]
    w_hh = np.asarray(inputs["w_hh"], np.float32)[GATE_PERM]
    bsum = (
        np.asarray(inputs["b_ih"], np.float32) + np.asarray(inputs["b_hh"], np.float32)
    )[GATE_PERM]
    wihT = np.ascontiguousarray(w_ih.T.reshape(2, 128, 4 * H).astype(ml_dtypes.bfloat16))
    whhT = np.ascontiguousarray(w_hh.T.reshape(2, 128, 4 * H).astype(ml_dtypes.bfloat16))
    biasg = np.ascontiguousarray(bsum.reshape(8, 128).T)

    w0 = np.ascontiguousarray(np.asarray(inputs["fc_w0"], np.float32).astype(ml_dtypes.bfloat16))
    b0t = np.ascontiguousarray(np.asarray(inputs["fc_b0"], np.float32).T)
    w1 = np.ascontiguousarray(np.asarray(inputs["fc_w1"], np.float32).astype(ml_dtypes.bfloat16))
    b1t = np.ascontiguousarray(np.asarray(inputs["fc_b1"], np.float32).T)
    affw = np.ascontiguousarray(np.asarray(inputs["affine_w"], np.float32).astype(ml_dtypes.bfloat16))
    affb = np.asarray(inputs["affine_b"], np.float32).reshape(1, 1)

    linear_w = np.asarray(inputs["linear_w"], np.float32)
    linT = np.ascontiguousarray(linear_w.T.reshape(2, 128, V).astype(ml_dtypes.bfloat16))

    # per-core valid-count profile; use max over cores so one program fits all
    counts_c = np.zeros((NCORES, T), np.int64)
    for c in range(NCORES):
        samples = c + NCORES * np.arange(BC)
        counts_c[c] = (length[samples][None, :] > np.arange(T)[:, None]).sum(1)
    cm = counts_c.max(0)

    in_maps = []
    meta = []
    for c in range(NCORES):
        samples = c + NCORES * np.arange(BC)  # descending lengths
        uidx_p = np.zeros((128, 1), np.int32)
        uidx_p[:BC, 0] = u_idx[samples]
        vidx_p = np.zeros((128, 1), np.int32)
        vidx_p[:BC, 0] = d_idx[samples] * NI + i_idx[samples]
        toks_tm = review[samples].T.reshape(T * BC)  # t-major
        ridx = np.ascontiguousarray(toks_tm.reshape(4, 128).T.astype(np.int32))
        dmask = np.zeros((1, D * BC), np.float32)
        for j, b in enumerate(samples):
            dmask[0, d_idx[b] * BC + j] = 1.0
        in_maps.append(
            {
                "uidx": uidx_p,
                "vidx": vidx_p,
                "ridx": ridx,
                "utab": utab,
                "itab": itab,
                "wtab": wtab,
                "wihT": wihT,
                "whhT": whhT,
                "biasg": biasg,
                "w0": w0,
                "b0t": b0t,
                "w1": w1,
                "b1t": b1t,
                "affw": affw,
                "affb": affb,
                "dmask": dmask,
                "linT": linT,
            }
        )
        meta.append((samples, length[samples]))
    return in_maps, meta, length, cm, np.asarray(inputs["linear_b"], np.float32)


def _assemble(results, meta, length, cm, linear_b):
    counts = (length[None, :] > np.arange(T)[:, None]).sum(1)  # global per-t
    cum = np.concatenate([[0], np.cumsum(counts)])
    pos = np.concatenate([[0], np.cumsum(cm)]).astype(int)
    P = int(cum[-1])
    outputs = np.empty((P, V), np.float32)
    rating = np.empty((B, 1), np.float32)
    for c in range(NCORES):
        samples, lens = meta[c]
        res = results[c]
        rating[samples, 0] = res["rating"][0]
        tt, jj = np.meshgrid(np.arange(T), np.arange(BC), indexing="ij")
        valid = tt < lens[jj]
        src = (pos[tt] + jj)[valid]
        dst = (cum[tt] + samples[jj])[valid]
        outputs[dst] = res["outp"][src]
    outputs += linear_b[None, :]
    return rating, outputs


def run(inputs, trace=False):
    in_maps, meta, length, cm, linear_b = _prep_inputs(inputs)
    nc = _get_nc(cm)
    res = run_bass_kernel_spmd(nc, in_maps, core_ids=list(range(NCORES)), trace=trace)
    rating, outputs = _assemble(res.results, meta, length, cm, linear_b)
    return rating, outputs, res


def kernel(**inputs):
    rating, outputs, _ = run(inputs, trace=False)
    return rating, outputs


# revision 14
# speedup vs baseline: 1.4618x; 1.2000x over previous
"""Trainium2 Bass kernel for nn_AutoGenReview_21114059227695 (moe_routing).

Strategy: fully data-parallel over batch B=128 across 8 NeuronCores
(16 samples per core, strided assignment for load balance since lengths
are sorted descending). Each core:
  1. gathers its user/item/word embedding rows on-device (indirect DMA)
  2. runs the domain-routed expert MLP (all 4 experts + one-hot select)
  3. runs the LSTM in feature-major layout (gates.T packed in PSUM; the
     x-part of the gates is bulk-precomputed and re-injected into PSUM
     through an identity matmul so ScalarE reads gates straight from PSUM)
  4. projects its packed hidden states onto the full vocab (row-parallel
     projection, bf16 weights resident in SBUF, fp32 accumulate),
     interleaved into the LSTM step loop as packed columns become ready
Host reassembles the time-major packed output rows and adds linear_b.
"""

import sys

for _p in ("/opt/trn_rl_repo",):
    if _p not in sys.path:
        sys.path.insert(0, _p)

import numpy as np
import ml_dtypes

import concourse.bass as bass
import concourse.tile as tile
from concourse import mybir
from concourse.bass_utils import run_bass_kernel_spmd
from concourse.masks import make_identity
from concourse.vector_clock import ScopedClock, VectorClock

F32 = mybir.dt.float32
BF16 = mybir.dt.bfloat16
I32 = mybir.dt.int32
AF = mybir.ActivationFunctionType

# problem constants
B, T = 128, 32
U, D, NI = 200000, 4, 50000
UD, ID = 64, 64
V, VW, H = 32000, 128, 256
NCORES = 8
BC = B // NCORES  # 16 samples per core
NV_CHUNK = 512  # vocab tile (psum bank limit, fp32)

# gate permutation: torch order [i f g o] -> kernel order [i f o g] so a
# single ScalarE sigmoid covers cols 0:96 and tanh covers 96:128
GATE_PERM = np.r_[0:256, 256:512, 768:1024, 512:768]


# ---------------------------------------------------------------------------
# Tile drain workaround: walrus on this image rejects >1 sem wait on the
# final TPB_CTRL drain; split the waits across a chain of SP nops.
def _patched_drain_and_barrier(self, tick_clock, wait_clock):
    nc = self.nc
    gc = tick_clock.global_clock
    nprocs = len(gc)
    for i in range(nprocs):
        t = gc[i]
        if t > 0:
            nop_inst = nc.sync.nop(nofuse=True, hint=f"drain_split_{i}")
            vc = VectorClock([0] * nprocs)
            vc.require_at_least(i, t)
            wait_clock.add_sem_waits(nop_inst.ins, ScopedClock({None: vc}))
    nc.sync.drain()
    nc.all_engine_barrier()
    assert self.sems is not None
    popped = nc._tile_sem_poison_stack.pop()
    assert popped is self._sem_poison
    nc.clear_and_free_semaphores(list(self.sems.allocated().values()))
    nc.all_engine_barrier()


tile.TileContext._drain_and_barrier = _patched_drain_and_barrier


def _split_multiwaits(nc):
    """walrus on this image encodes at most one sem-wait per instruction;
    hoist extra waits onto single-wait NoOps immediately preceding the
    instruction on the same engine (in-order sequencers make this
    semantically identical)."""
    fn = nc.m.functions[0]
    ctr = 0
    for b in fn.blocks:
        out = []
        changed = False
        for inst in b.instructions:
            si = inst.sync_info
            if si is not None and len(si.on_wait) > 1:
                changed = True
                waits = list(si.on_wait)
                for w in waits[:-1]:
                    ctr += 1
                    out.append(
                        mybir.InstNoOp(
                            name=f"WSPLIT-{ctr}",
                            engine=inst.engine,
                            sync_info=mybir.SyncInfo(on_wait=[w], on_update=[]),
                        )
                    )
                inst.sync_info = mybir.SyncInfo(
                    on_wait=[waits[-1]], on_update=list(si.on_update)
                )
            out.append(inst)
        if changed:
            b.instructions = out


# ---------------------------------------------------------------------------
def build_program(cm):
    """cm: per-timestep packed column width (max over cores of per-core
    valid-sample counts), non-increasing, cm[0] == BC."""
    cm = list(cm)
    pos = np.concatenate([[0], np.cumsum(cm)]).astype(int)  # packed offsets
    PP = int(pos[-1])  # packed columns per core
    nmc = (PP + 127) // 128  # projection row chunks

    nc = bass.Bass()

    d_uidx = nc.declare_dram_parameter("uidx", [128, 1], I32, isOutput=False)
    d_vidx = nc.declare_dram_parameter("vidx", [128, 1], I32, isOutput=False)
    d_ridx = nc.declare_dram_parameter("ridx", [128, 4], I32, isOutput=False)
    d_utab = nc.declare_dram_parameter("utab", [U, UD], F32, isOutput=False)
    d_itab = nc.declare_dram_parameter("itab", [D * NI, ID], F32, isOutput=False)
    d_wtab = nc.declare_dram_parameter("wtab", [V, VW], F32, isOutput=False)
    d_wih = nc.declare_dram_parameter("wihT", [2, 128, 4 * H], BF16, isOutput=False)
    d_whh = nc.declare_dram_parameter("whhT", [2, 128, 4 * H], BF16, isOutput=False)
    d_bias = nc.declare_dram_parameter("biasg", [128, 8], F32, isOutput=False)
    d_w0 = nc.declare_dram_parameter("w0", [D, 128, 128], BF16, isOutput=False)
    d_b0 = nc.declare_dram_parameter("b0t", [128, D], F32, isOutput=False)
    d_w1 = nc.declare_dram_parameter("w1", [D, 128, 64], BF16, isOutput=False)
    d_b1 = nc.declare_dram_parameter("b1t", [64, D], F32, isOutput=False)
    d_aw = nc.declare_dram_parameter("affw", [64, 1], BF16, isOutput=False)
    d_ab = nc.declare_dram_parameter("affb", [1, 1], F32, isOutput=False)
    d_mask = nc.declare_dram_parameter("dmask", [1, D * BC], F32, isOutput=False)
    d_lin = nc.declare_dram_parameter("linT", [2, 128, V], BF16, isOutput=False)
    d_out = nc.declare_dram_parameter("outp", [PP, V], F32, isOutput=True)
    d_rate = nc.declare_dram_parameter("rating", [1, BC], F32, isOutput=True)

    nfull, nrem = divmod(V, NV_CHUNK)
    nchunks = nfull + (1 if nrem else 0)

    with tile.TileContext(nc) as tc:
        with (
            tc.tile_pool(name="const", bufs=1) as cpool,
            tc.tile_pool(name="work", bufs=2) as wpool,
            tc.tile_pool(name="big", bufs=1) as bigpool,
            tc.tile_pool(name="ob", bufs=4) as obpool,
            tc.tile_pool(name="ps", bufs=2, space="PSUM") as pspool,
            tc.tile_pool(name="psproj", bufs=3, space="PSUM") as psproj,
            tc.tile_pool(name="psg", bufs=2, space="PSUM") as psg_pool,
        ):
            # ---- small loads first: indices, LSTM/expert weights ----
            sb_uidx = cpool.tile([128, 1], I32, tag="uidx")
            nc.sync.dma_start(out=sb_uidx[:], in_=d_uidx[:])
            sb_vidx = cpool.tile([128, 1], I32, tag="vidx")
            nc.sync.dma_start(out=sb_vidx[:], in_=d_vidx[:])
            sb_ridx = cpool.tile([128, 4], I32, tag="ridx")
            nc.sync.dma_start(out=sb_ridx[:], in_=d_ridx[:])

            sb_wih = cpool.tile([128, 2, 4 * H], BF16, tag="wih")
            nc.sync.dma_start(out=sb_wih[:], in_=d_wih[:].rearrange("k p n -> p k n"))
            sb_whh = cpool.tile([128, 2, 4 * H], BF16, tag="whh")
            nc.sync.dma_start(out=sb_whh[:], in_=d_whh[:].rearrange("k p n -> p k n"))
            sb_bias = cpool.tile([128, 8], F32, tag="biasg")
            nc.sync.dma_start(out=sb_bias[:], in_=d_bias[:])

            sb_w0 = cpool.tile([128, D, 128], BF16, tag="w0")
            nc.scalar.dma_start(out=sb_w0[:], in_=d_w0[:].rearrange("d k m -> k d m"))
            sb_b0 = cpool.tile([128, D], F32, tag="b0")
            nc.scalar.dma_start(out=sb_b0[:], in_=d_b0[:])
            sb_w1 = cpool.tile([128, D, 64], BF16, tag="w1")
            nc.scalar.dma_start(out=sb_w1[:], in_=d_w1[:].rearrange("d k m -> k d m"))
            sb_b1 = cpool.tile([64, D], F32, tag="b1")
            nc.scalar.dma_start(out=sb_b1[:], in_=d_b1[:])
            sb_aw = cpool.tile([64, 1], BF16, tag="affw")
            nc.scalar.dma_start(out=sb_aw[:], in_=d_aw[:])
            sb_ab = cpool.tile([1, 1], F32, tag="affb")
            nc.scalar.dma_start(out=sb_ab[:], in_=d_ab[:])
            sb_mask = cpool.tile([1, D * BC], F32, tag="dmask")
            nc.scalar.dma_start(out=sb_mask[:], in_=d_mask[:])

            ident = cpool.tile([128, 128], F32, tag="ident")
            make_identity(nc, ident[:])
            ident_b = cpool.tile([128, 128], BF16, tag="identb")
            nc.vector.tensor_copy(ident_b[:], ident[:])

            # ---- embedding gathers (indirect DMA: one row per partition) ----
            uv_sb = wpool.tile([128, 128], F32, tag="uvrows")
            nc.gpsimd.indirect_dma_start(
                out=uv_sb[:, 0:UD],
                out_offset=None,
                in_=d_utab[:],
                in_offset=bass.IndirectOffsetOnAxis(ap=sb_uidx[:, :1], axis=0),
            )
            nc.gpsimd.indirect_dma_start(
                out=uv_sb[:, UD : UD + ID],
                out_offset=None,
                in_=d_itab[:],
                in_offset=bass.IndirectOffsetOnAxis(ap=sb_vidx[:, :1], axis=0),
            )

            ps_t = pspool.tile([128, 128], F32, tag="scratch")
            nc.tensor.transpose(out=ps_t[:], in_=uv_sb[:], identity=ident[:])
            uvT_b = wpool.tile([128, BC], BF16, tag="uvT_b")
            nc.vector.tensor_copy(uvT_b[:], ps_t[:, 0:BC])

            wT = bigpool.tile([128, T * BC], BF16, tag="wT")
            for g in range(4):
                wg = wpool.tile([128, 128], F32, tag="wg")
                nc.gpsimd.indirect_dma_start(
                    out=wg[:],
                    out_offset=None,
                    in_=d_wtab[:],
                    in_offset=bass.IndirectOffsetOnAxis(ap=sb_ridx[:, g : g + 1], axis=0),
                )
                ps_w = pspool.tile([128, 128], F32, tag="scratch")
                nc.tensor.transpose(out=ps_w[:], in_=wg[:], identity=ident[:])
                nc.vector.tensor_copy(wT[:, g * 128 : (g + 1) * 128], ps_w[:])

            # ---- LSTM bulk x-part (bf16 gxp, packed gate layout) ----
            ps_guv = psg_pool.tile([128, 128], F32, tag="gate")
            for m in range(8):
                nc.tensor.matmul(
                    out=ps_guv[:, m * BC : (m + 1) * BC],
                    lhsT=sb_wih[:, 0, m * 128 : (m + 1) * 128],
                    rhs=uvT_b[:],
                    start=True,
                    stop=True,
                )
            sbias = wpool.tile([128, 128], F32, tag="sbias")
            for m in range(8):
                nc.scalar.activation(
                    out=sbias[:, m * BC : (m + 1) * BC],
                    in_=ps_guv[:, m * BC : (m + 1) * BC],
                    func=AF.Identity,
                    bias=sb_bias[:, m : m + 1],
                )
            gxp = bigpool.tile([128, T * 128], BF16, tag="gxp")
            gxp_v = gxp[:].rearrange("p (t m j) -> p m j t", t=T, m=8)
            for m in range(8):
                ps_b = pspool.tile([128, 512], F32, tag="scratch")
                nc.tensor.matmul(
                    out=ps_b[:],
                    lhsT=sb_wih[:, 1, m * 128 : (m + 1) * 128],
                    rhs=wT[:],
                    start=True,
                    stop=True,
                )
                nc.vector.tensor_add(
                    out=gxp_v[:, m],
                    in0=ps_b[:].rearrange("p (t j) -> p j t", t=T),
                    in1=sbias[:, m * BC : (m + 1) * BC].to_broadcast([128, BC, T]),
                )

            # ---- resident linear_w.T (bf16, 125KB/partition), low cols first ----
            lin_sb = cpool.tile([128, 2, V], BF16, tag="lin")
            lin_src = d_lin[:].rearrange("k p n -> p k n")
            dma_engines = [nc.sync, nc.scalar, nc.gpsimd]
            for s in range(8):
                sl = slice(s * (V // 8), (s + 1) * (V // 8))
                dma_engines[s % 3].dma_start(
                    out=lin_sb[:, :, sl], in_=lin_src[:, :, sl]
                )

            # ---- LSTM recurrence + interleaved projection ----
            # hsP[p, k*PP + pos[t] + j] = h_t[k*128+p, j]  (bf16, packed cols)
            hsP = bigpool.tile([128, 2 * PP], BF16, tag="hsP")
            hsP_k = hsP[:].rearrange("p (k m) -> p k m", k=2)
            c_bufs = [
                bigpool.tile([128, 2 * BC], F32, name=f"cbuf{i}", tag=f"cbuf{i}")
                for i in range(2)
            ]

            nmc_full = PP // 128  # full 128-row projection chunks
            mtail = PP - nmc_full * 128  # leftover rows (col-tiled tail)
            tasks = []  # (mc, n)
            emitted_chunks = [False] * nmc_full
            ob_engines = [nc.sync, nc.gpsimd]
            ob_ctr = [0]

            def emit_proj(ntasks):
                for _ in range(ntasks):
                    if not tasks:
                        return
                    mc, n = tasks.pop(0)
                    nn_ = NV_CHUNK if n < nfull else nrem
                    nsl = slice(n * NV_CHUNK, n * NV_CHUNK + nn_)
                    ps_p = psproj.tile([128, NV_CHUNK], F32, tag="proj")
                    for k in range(2):
                        nc.tensor.matmul(
                            out=ps_p[:, 0:nn_],
                            lhsT=hsP_k[:, k, mc * 128 : (mc + 1) * 128],
                            rhs=lin_sb[:, k, nsl],
                            start=(k == 0),
                            stop=(k == 1),
                        )
                    ob = obpool.tile([128, NV_CHUNK], F32, tag="ob")
                    if ob_ctr[0] % 3 == 2:
                        nc.scalar.copy(ob[:, 0:nn_], ps_p[:, 0:nn_])
                    else:
                        nc.vector.tensor_copy(ob[:, 0:nn_], ps_p[:, 0:nn_])
                    eng = ob_engines[ob_ctr[0] % 2]
                    ob_ctr[0] += 1
                    eng.dma_start(
                        out=d_out[mc * 128 : (mc + 1) * 128, nsl],
                        in_=ob[:, 0:nn_],
                    )

            for t in range(T):
                w = cm[t]
                ps_g = psg_pool.tile([128, 128], F32, tag="gate")
                # inject precomputed x-part gates into PSUM via identity matmul
                nc.tensor.matmul(
                    out=ps_g[:],
                    lhsT=ident_b[:],
                    rhs=gxp[:, t * 128 : (t + 1) * 128],
                    start=True,
                    stop=(t == 0),
                    skip_group_check=True,
                )
                if t > 0:
                    for m in range(8):
                        for k in range(2):
                            nc.tensor.matmul(
                                out=ps_g[:, m * BC : m * BC + w],
                                lhsT=sb_whh[:, k, m * 128 : (m + 1) * 128],
                                rhs=hsP_k[:, k, pos[t - 1] : pos[t - 1] + w],
                                start=False,
                                stop=(m == 7 and k == 1),
                                skip_group_check=True,
                            )
                A = wpool.tile([128, 128], F32, tag="At")
                nc.scalar.activation(out=A[:, 0:96], in_=ps_g[:, 0:96], func=AF.Sigmoid)
                nc.scalar.activation(out=A[:, 96:128], in_=ps_g[:, 96:128], func=AF.Tanh)
                t1 = wpool.tile([128, 2 * BC], F32, tag="t1")
                nc.vector.tensor_mul(t1[:], A[:, 0:32], A[:, 96:128])
                c_new = c_bufs[t % 2]
                if t == 0:
                    nc.vector.tensor_copy(c_new[:], t1[:])
                else:
                    c_old = c_bufs[(t - 1) % 2]
                    nc.vector.tensor_mul(c_new[:], A[:, 32:64], c_old[:])
                    nc.vector.tensor_add(c_new[:], c_new[:], t1[:])
                tct = wpool.tile([128, 2 * BC], F32, tag="tct")
                nc.scalar.activation(out=tct[:], in_=c_new[:], func=AF.Tanh)
                # write h packed (only the first cm[t] columns are kept)
                hw = hsP[:].rearrange("p (k m) -> p k m", k=2)[
                    :, :, pos[t] : pos[t] + w
                ]
                nc.vector.tensor_mul(
                    out=hw,
                    in0=A[:, 64:96].rearrange("p (k j) -> p k j", k=2)[:, :, 0:w],
                    in1=tct[:].rearrange("p (k j) -> p k j", k=2)[:, :, 0:w],
                )
                # release projection tasks for chunks fully written
                for mc in range(nmc_full):
                    if not emitted_chunks[mc] and pos[t + 1] >= (mc + 1) * 128:
                        emitted_chunks[mc] = True
                        tasks.extend((mc, n) for n in range(nchunks))
                if t >= 1 and t < T - 1:
                    budget = int(np.ceil(len(tasks) / max(1, T - 1 - t)))
                    emit_proj(budget)
            emit_proj(len(tasks) + 1)

            # ---- projection tail ----
            if mtail > 32:
                # plain row-chunk tail (rare: only when >32 leftover rows)
                for n in range(nchunks):
                    nn_ = NV_CHUNK if n < nfull else nrem
                    nsl = slice(n * NV_CHUNK, n * NV_CHUNK + nn_)
                    ps_p = psproj.tile([128, NV_CHUNK], F32, tag="proj")
                    for k in range(2):
                        nc.tensor.matmul(
                            out=ps_p[0:mtail, 0:nn_],
                            lhsT=hsP_k[:, k, nmc_full * 128 : PP],
                            rhs=lin_sb[:, k, nsl],
                            start=(k == 0),
                            stop=(k == 1),
                        )
                    ob = obpool.tile([128, NV_CHUNK], F32, tag="ob")
                    nc.vector.tensor_copy(ob[0:mtail, 0:nn_], ps_p[0:mtail, 0:nn_])
                    ob_engines[n % 2].dma_start(
                        out=d_out[nmc_full * 128 : PP, nsl], in_=ob[0:mtail, 0:nn_]
                    )
            # col-tiled tail: mtail<=32 rows packed into the 4 PE col-groups
            elif mtail > 0:
                for g in range((nchunks + 3) // 4):
                    ns = [n for n in (4 * g + j for j in range(4)) if n < nchunks]
                    ps_p = psproj.tile([128, NV_CHUNK], F32, tag="proj")
                    for j, n in enumerate(ns):
                        nn_ = NV_CHUNK if n < nfull else nrem
                        nsl = slice(n * NV_CHUNK, n * NV_CHUNK + nn_)
                        for k in range(2):
                            nc.tensor.matmul(
                                out=ps_p[32 * j : 32 * j + mtail, 0:nn_],
                                lhsT=hsP_k[:, k, nmc_full * 128 : PP],
                                rhs=lin_sb[:, k, nsl],
                                tile_position=(0, 32 * j),
                                start=(k == 0),
                                stop=(k == 1),
                                skip_group_check=True,
                            )
                    ob = obpool.tile([128, NV_CHUNK], F32, tag="ob")
                    if g % 3 == 2:
                        nc.scalar.copy(ob[:], ps_p[:])
                    else:
                        nc.vector.tensor_copy(ob[:], ps_p[:])
                    for j, n in enumerate(ns):
                        nn_ = NV_CHUNK if n < nfull else nrem
                        nsl = slice(n * NV_CHUNK, n * NV_CHUNK + nn_)
                        ob_engines[j % 2].dma_start(
                            out=d_out[nmc_full * 128 : PP, nsl],
                            in_=ob[32 * j : 32 * j + mtail, 0:nn_],
                        )

            # ---- expert MLP (all 4 domains, one-hot select) ----
            ps_e1 = pspool.tile([128, D * BC], F32, tag="scratch")
            h1 = wpool.tile([128, D * BC], BF16, tag="h1")
            for d in range(D):
                nc.tensor.matmul(
                    out=ps_e1[:, d * BC : (d + 1) * BC],
                    lhsT=sb_w0[:, d, :],
                    rhs=uvT_b[:],
                    start=True,
                    stop=True,
                )
            for d in range(D):
                nc.scalar.activation(
                    out=h1[:, d * BC : (d + 1) * BC],
                    in_=ps_e1[:, d * BC : (d + 1) * BC],
                    func=AF.Relu,
                    bias=sb_b0[:, d : d + 1],
                )
            ps_e2 = pspool.tile([64, D * BC], F32, tag="scratch")
            h2 = wpool.tile([64, D * BC], BF16, tag="h2")
            for d in range(D):
                nc.tensor.matmul(
                    out=ps_e2[:, d * BC : (d + 1) * BC],
                    lhsT=sb_w1[:, d, :],
                    rhs=h1[:, d * BC : (d + 1) * BC],
                    start=True,
                    stop=True,
                )
            for d in range(D):
                nc.scalar.activation(
                    out=h2[:, d * BC : (d + 1) * BC],
                    in_=ps_e2[:, d * BC : (d + 1) * BC],
                    func=AF.Relu,
                    bias=sb_b1[:, d : d + 1],
                )
            ps_e3 = pspool.tile([1, D * BC], F32, tag="scratch")
            for d in range(D):
                nc.tensor.matmul(
                    out=ps_e3[:, d * BC : (d + 1) * BC],
                    lhsT=sb_aw[:],
                    rhs=h2[:, d * BC : (d + 1) * BC],
                    start=True,
                    stop=True,
                )
            r_sig = wpool.tile([1, D * BC], F32, tag="r_sig")
            nc.scalar.activation(out=r_sig[:], in_=ps_e3[:], func=AF.Sigmoid, bias=sb_ab[0:1, 0:1])
            r5 = wpool.tile([1, D * BC], F32, tag="r5")
            nc.vector.tensor_scalar_mul(r5[:], r_sig[:], 5.0)
            nc.vector.tensor_mul(r5[:], r5[:], sb_mask[:])
            r_fin = wpool.tile([1, BC], F32, tag="r_fin")
            nc.vector.tensor_add(r_fin[:], r5[:, 0:BC], r5[:, BC : 2 * BC])
            nc.vector.tensor_add(r_fin[:], r_fin[:], r5[:, 2 * BC : 3 * BC])
            nc.vector.tensor_add(r_fin[:], r_fin[:], r5[:, 3 * BC : 4 * BC])
            nc.sync.dma_start(out=d_rate[:], in_=r_fin[:])

    _split_multiwaits(nc)
    return nc


_NC_CACHE = {}


def _get_nc(cm):
    key = tuple(cm)
    if key not in _NC_CACHE:
        _NC_CACHE[key] = build_program(cm)
    return _NC_CACHE[key]


def _prep_inputs(inputs):
    """Build the 8 per-core in_maps + host-side assembly metadata."""
    u_idx = np.asarray(inputs["u_idx"]).astype(np.int32)
    i_idx = np.asarray(inputs["i_idx"]).astype(np.int32)
    d_idx = np.asarray(inputs["d_idx"]).astype(np.int32)
    review = np.asarray(inputs["review"]).astype(np.int32)
    length = np.asarray(inputs["length"]).astype(np.int32)

    utab = np.ascontiguousarray(np.asarray(inputs["emb_users_w"], np.float32))
    itab = np.ascontiguousarray(
        np.asarray(inputs["emb_items_w"], np.float32).reshape(D * NI, ID)
    )
    wtab = np.ascontiguousarray(np.asarray(inputs["word_emb_w"], np.float32))

    w_ih = np.asarray(inputs["w_ih"], np.float32)[GATE_PERM]
    w_hh = np.asarray(inputs["w_hh"], np.float32)[GATE_PERM]
    bsum = (
        np.asarray(inputs["b_ih"], np.float32) + np.asarray(inputs["b_hh"], np.float32)
    )[GATE_PERM]
    wihT = np.ascontiguousarray(w_ih.T.reshape(2, 128, 4 * H).astype(ml_dtypes.bfloat16))
    whhT = np.ascontiguousarray(w_hh.T.reshape(2, 128, 4 * H).astype(ml_dtypes.bfloat16))
    biasg = np.ascontiguousarray(bsum.reshape(8, 128).T)

    w0 = np.ascontiguousarray(np.asarray(inputs["fc_w0"], np.float32).astype(ml_dtypes.bfloat16))
    b0t = np.ascontiguousarray(np.asarray(inputs["fc_b0"], np.float32).T)
    w1 = np.ascontiguousarray(np.asarray(inputs["fc_w1"], np.float32).astype(ml_dtypes.bfloat16))
    b1t = np.ascontiguousarray(np.asarray(inputs["fc_b1"], np.float32).T)
    affw = np.ascontiguousarray(np.asarray(inputs["affine_w"], np.float32).astype(ml_dtypes.bfloat16))
    affb = np.asarray(inputs["affine_b"], np.float32).reshape(1, 1)

    linear_w = np.asarray(inputs["linear_w"], np.float32)
    linT = np.ascontiguousarray(linear_w.T.reshape(2, 128, V).astype(ml_dtypes.bfloat16))

    # per-core valid-count profile; use max over cores so one program fits all
    counts_c = np.zeros((NCORES, T), np.int64)
    for c in range(NCORES):
        samples = c + NCORES * np.arange(BC)
        counts_c[c] = (length[samples][None, :] > np.arange(T)[:, None]).sum(1)
    cm = counts_c.max(0)

    in_maps = []
    meta = []
    for c in range(NCORES):
        samples = c + NCORES * np.arange(BC)  # descending lengths
        uidx_p = np.zeros((128, 1), np.int32)
        uidx_p[:BC, 0] = u_idx[samples]
        vidx_p = np.zeros((128, 1), np.int32)
        vidx_p[:BC, 0] = d_idx[samples] * NI + i_idx[samples]
        toks_tm = review[samples].T.reshape(T * BC)  # t-major
        ridx = np.ascontiguousarray(toks_tm.reshape(4, 128).T.astype(np.int32))
        dmask = np.zeros((1, D * BC), np.float32)
        for j, b in enumerate(samples):
            dmask[0, d_idx[b] * BC + j] = 1.0
        in_maps.append(
            {
                "uidx": uidx_p,
                "vidx": vidx_p,
                "ridx": ridx,
                "utab": utab,
                "itab": itab,
                "wtab": wtab,
                "wihT": wihT,
                "whhT": whhT,
                "biasg": biasg,
                "w0": w0,
                "b0t": b0t,
                "w1": w1,
                "b1t": b1t,
                "affw": affw,
                "affb": affb,
                "dmask": dmask,
                "linT": linT,
            }
        )
        meta.append((samples, length[samples]))
    return in_maps, meta, length, cm, np.asarray(inputs["linear_b"], np.float32)


def _assemble(results, meta, length, cm, linear_b):
    counts = (length[None, :] > np.arange(T)[:, None]).sum(1)  # global per-t
    cum = np.concatenate([[0], np.cumsum(counts)])
    pos = np.concatenate([[0], np.cumsum(cm)]).astype(int)
    P = int(cum[-1])
    outputs = np.empty((P, V), np.float32)
    rating = np.empty((B, 1), np.float32)
    for c in range(NCORES):
        samples, lens = meta[c]
        res = results[c]
        rating[samples, 0] = res["rating"][0]
        tt, jj = np.meshgrid(np.arange(T), np.arange(BC), indexing="ij")
        valid = tt < lens[jj]
        src = (pos[tt] + jj)[valid]
        dst = (cum[tt] + samples[jj])[valid]
        outputs[dst] = res["outp"][src]
    outputs += linear_b[None, :]
    return rating, outputs


def run(inputs, trace=False):
    in_maps, meta, length, cm, linear_b = _prep_inputs(inputs)
    nc = _get_nc(cm)
    res = run_bass_kernel_spmd(nc, in_maps, core_ids=list(range(NCORES)), trace=trace)
    rating, outputs = _assemble(res.results, meta, length, cm, linear_b)
    return rating, outputs, res


def kernel(**inputs):
    rating, outputs, _ = run(inputs, trace=False)
    return rating, outputs


# revision 15
# speedup vs baseline: 1.4797x; 1.0123x over previous
"""Trainium2 Bass kernel for nn_AutoGenReview_21114059227695 (moe_routing).

Strategy: fully data-parallel over batch B=128 across 8 NeuronCores
(16 samples per core, strided assignment for load balance since lengths
are sorted descending). Each core:
  1. gathers its user/item/word embedding rows on-device (indirect DMA)
  2. runs the domain-routed expert MLP (all 4 experts + one-hot select)
  3. runs the LSTM in feature-major layout (gates.T packed in PSUM; the
     x-part of the gates is bulk-precomputed and re-injected into PSUM
     through an identity matmul so ScalarE reads gates straight from PSUM)
  4. projects its packed hidden states onto the full vocab (row-parallel
     projection, bf16 weights resident in SBUF, fp32 accumulate),
     interleaved into the LSTM step loop as packed columns become ready
Host reassembles the time-major packed output rows and adds linear_b.
"""

import sys

for _p in ("/opt/trn_rl_repo",):
    if _p not in sys.path:
        sys.path.insert(0, _p)

import numpy as np
import ml_dtypes

import concourse.bass as bass
import concourse.tile as tile
from concourse import mybir
from concourse.bass_utils import run_bass_kernel_spmd
from concourse.masks import make_identity
from concourse.vector_clock import ScopedClock, VectorClock

F32 = mybir.dt.float32
BF16 = mybir.dt.bfloat16
I32 = mybir.dt.int32
AF = mybir.ActivationFunctionType

# problem constants
B, T = 128, 32
U, D, NI = 200000, 4, 50000
UD, ID = 64, 64
V, VW, H = 32000, 128, 256
NCORES = 8
BC = B // NCORES  # 16 samples per core
NV_CHUNK = 512  # vocab tile (psum bank limit, fp32)

# gate permutation: torch order [i f g o] -> kernel order [i f o g] so a
# single ScalarE sigmoid covers cols 0:96 and tanh covers 96:128
GATE_PERM = np.r_[0:256, 256:512, 768:1024, 512:768]


# ---------------------------------------------------------------------------
# Tile drain workaround: walrus on this image rejects >1 sem wait on the
# final TPB_CTRL drain; split the waits across a chain of SP nops.
def _patched_drain_and_barrier(self, tick_clock, wait_clock):
    nc = self.nc
    gc = tick_clock.global_clock
    nprocs = len(gc)
    for i in range(nprocs):
        t = gc[i]
        if t > 0:
            nop_inst = nc.sync.nop(nofuse=True, hint=f"drain_split_{i}")
            vc = VectorClock([0] * nprocs)
            vc.require_at_least(i, t)
            wait_clock.add_sem_waits(nop_inst.ins, ScopedClock({None: vc}))
    nc.sync.drain()
    nc.all_engine_barrier()
    assert self.sems is not None
    popped = nc._tile_sem_poison_stack.pop()
    assert popped is self._sem_poison
    nc.clear_and_free_semaphores(list(self.sems.allocated().values()))
    nc.all_engine_barrier()


tile.TileContext._drain_and_barrier = _patched_drain_and_barrier


def _split_multiwaits(nc):
    """walrus on this image encodes at most one sem-wait per instruction;
    hoist extra waits onto single-wait NoOps immediately preceding the
    instruction on the same engine (in-order sequencers make this
    semantically identical)."""
    fn = nc.m.functions[0]
    ctr = 0
    for b in fn.blocks:
        out = []
        changed = False
        for inst in b.instructions:
            si = inst.sync_info
            if si is not None and len(si.on_wait) > 1:
                changed = True
                waits = list(si.on_wait)
                for w in waits[:-1]:
                    ctr += 1
                    out.append(
                        mybir.InstNoOp(
                            name=f"WSPLIT-{ctr}",
                            engine=inst.engine,
                            sync_info=mybir.SyncInfo(on_wait=[w], on_update=[]),
                        )
                    )
                inst.sync_info = mybir.SyncInfo(
                    on_wait=[waits[-1]], on_update=list(si.on_update)
                )
            out.append(inst)
        if changed:
            b.instructions = out


# ---------------------------------------------------------------------------
def build_program(cm):
    """cm: per-timestep packed column width (max over cores of per-core
    valid-sample counts), non-increasing, cm[0] == BC."""
    cm = list(cm)
    pos = np.concatenate([[0], np.cumsum(cm)]).astype(int)  # packed offsets
    PP = int(pos[-1])  # packed columns per core
    nmc = (PP + 127) // 128  # projection row chunks

    nc = bass.Bass()

    d_uidx = nc.declare_dram_parameter("uidx", [128, 1], I32, isOutput=False)
    d_vidx = nc.declare_dram_parameter("vidx", [128, 1], I32, isOutput=False)
    d_ridx = nc.declare_dram_parameter("ridx", [128, 4], I32, isOutput=False)
    d_utab = nc.declare_dram_parameter("utab", [U, UD], F32, isOutput=False)
    d_itab = nc.declare_dram_parameter("itab", [D * NI, ID], F32, isOutput=False)
    d_wtab = nc.declare_dram_parameter("wtab", [V, VW], F32, isOutput=False)
    d_wih = nc.declare_dram_parameter("wihT", [2, 128, 4 * H], BF16, isOutput=False)
    d_whh = nc.declare_dram_parameter("whhT", [2, 128, 4 * H], BF16, isOutput=False)
    d_bias = nc.declare_dram_parameter("biasg", [128, 8], F32, isOutput=False)
    d_w0 = nc.declare_dram_parameter("w0", [D, 128, 128], BF16, isOutput=False)
    d_b0 = nc.declare_dram_parameter("b0t", [128, D], F32, isOutput=False)
    d_w1 = nc.declare_dram_parameter("w1", [D, 128, 64], BF16, isOutput=False)
    d_b1 = nc.declare_dram_parameter("b1t", [64, D], F32, isOutput=False)
    d_aw = nc.declare_dram_parameter("affw", [64, 1], BF16, isOutput=False)
    d_ab = nc.declare_dram_parameter("affb", [1, 1], F32, isOutput=False)
    d_mask = nc.declare_dram_parameter("dmask", [1, D * BC], F32, isOutput=False)
    d_lin = nc.declare_dram_parameter("linT", [2, 128, V], BF16, isOutput=False)
    d_out = nc.declare_dram_parameter("outp", [PP, V], F32, isOutput=True)
    d_rate = nc.declare_dram_parameter("rating", [1, BC], F32, isOutput=True)

    nfull, nrem = divmod(V, NV_CHUNK)
    nchunks = nfull + (1 if nrem else 0)

    with tile.TileContext(nc) as tc:
        with (
            tc.tile_pool(name="const", bufs=1) as cpool,
            tc.tile_pool(name="work", bufs=2) as wpool,
            tc.tile_pool(name="big", bufs=1) as bigpool,
            tc.tile_pool(name="ob", bufs=4) as obpool,
            tc.tile_pool(name="ps", bufs=2, space="PSUM") as pspool,
            tc.tile_pool(name="psproj", bufs=4, space="PSUM") as psproj,
            tc.tile_pool(name="psg", bufs=2, space="PSUM") as psg_pool,
        ):
            # ---- small loads first: indices, LSTM/expert weights ----
            sb_uidx = cpool.tile([128, 1], I32, tag="uidx")
            nc.sync.dma_start(out=sb_uidx[:], in_=d_uidx[:])
            sb_vidx = cpool.tile([128, 1], I32, tag="vidx")
            nc.sync.dma_start(out=sb_vidx[:], in_=d_vidx[:])
            sb_ridx = cpool.tile([128, 4], I32, tag="ridx")
            nc.sync.dma_start(out=sb_ridx[:], in_=d_ridx[:])

            sb_wih = cpool.tile([128, 2, 4 * H], BF16, tag="wih")
            nc.sync.dma_start(out=sb_wih[:], in_=d_wih[:].rearrange("k p n -> p k n"))
            sb_whh = cpool.tile([128, 2, 4 * H], BF16, tag="whh")
            nc.sync.dma_start(out=sb_whh[:], in_=d_whh[:].rearrange("k p n -> p k n"))
            sb_bias = cpool.tile([128, 8], F32, tag="biasg")
            nc.sync.dma_start(out=sb_bias[:], in_=d_bias[:])

            sb_w0 = cpool.tile([128, D, 128], BF16, tag="w0")
            nc.scalar.dma_start(out=sb_w0[:], in_=d_w0[:].rearrange("d k m -> k d m"))
            sb_b0 = cpool.tile([128, D], F32, tag="b0")
            nc.scalar.dma_start(out=sb_b0[:], in_=d_b0[:])
            sb_w1 = cpool.tile([128, D, 64], BF16, tag="w1")
            nc.scalar.dma_start(out=sb_w1[:], in_=d_w1[:].rearrange("d k m -> k d m"))
            sb_b1 = cpool.tile([64, D], F32, tag="b1")
            nc.scalar.dma_start(out=sb_b1[:], in_=d_b1[:])
            sb_aw = cpool.tile([64, 1], BF16, tag="affw")
            nc.scalar.dma_start(out=sb_aw[:], in_=d_aw[:])
            sb_ab = cpool.tile([1, 1], F32, tag="affb")
            nc.scalar.dma_start(out=sb_ab[:], in_=d_ab[:])
            sb_mask = cpool.tile([1, D * BC], F32, tag="dmask")
            nc.scalar.dma_start(out=sb_mask[:], in_=d_mask[:])

            ident = cpool.tile([128, 128], F32, tag="ident")
            make_identity(nc, ident[:])
            ident_b = cpool.tile([128, 128], BF16, tag="identb")
            nc.vector.tensor_copy(ident_b[:], ident[:])

            # ---- embedding gathers (indirect DMA: one row per partition) ----
            uv_sb = wpool.tile([128, 128], F32, tag="uvrows")
            nc.gpsimd.indirect_dma_start(
                out=uv_sb[:, 0:UD],
                out_offset=None,
                in_=d_utab[:],
                in_offset=bass.IndirectOffsetOnAxis(ap=sb_uidx[:, :1], axis=0),
            )
            nc.gpsimd.indirect_dma_start(
                out=uv_sb[:, UD : UD + ID],
                out_offset=None,
                in_=d_itab[:],
                in_offset=bass.IndirectOffsetOnAxis(ap=sb_vidx[:, :1], axis=0),
            )

            ps_t = pspool.tile([128, 128], F32, tag="scratch")
            nc.tensor.transpose(out=ps_t[:], in_=uv_sb[:], identity=ident[:])
            uvT_b = wpool.tile([128, BC], BF16, tag="uvT_b")
            nc.vector.tensor_copy(uvT_b[:], ps_t[:, 0:BC])

            wT = bigpool.tile([128, T * BC], BF16, tag="wT")
            for g in range(4):
                wg = wpool.tile([128, 128], F32, tag="wg")
                nc.gpsimd.indirect_dma_start(
                    out=wg[:],
                    out_offset=None,
                    in_=d_wtab[:],
                    in_offset=bass.IndirectOffsetOnAxis(ap=sb_ridx[:, g : g + 1], axis=0),
                )
                ps_w = pspool.tile([128, 128], F32, tag="scratch")
                nc.tensor.transpose(out=ps_w[:], in_=wg[:], identity=ident[:])
                nc.vector.tensor_copy(wT[:, g * 128 : (g + 1) * 128], ps_w[:])

            # ---- LSTM bulk x-part (bf16 gxp, packed gate layout) ----
            ps_guv = psg_pool.tile([128, 128], F32, tag="gate")
            for m in range(8):
                nc.tensor.matmul(
                    out=ps_guv[:, m * BC : (m + 1) * BC],
                    lhsT=sb_wih[:, 0, m * 128 : (m + 1) * 128],
                    rhs=uvT_b[:],
                    start=True,
                    stop=True,
                )
            sbias = wpool.tile([128, 128], F32, tag="sbias")
            for m in range(8):
                nc.scalar.activation(
                    out=sbias[:, m * BC : (m + 1) * BC],
                    in_=ps_guv[:, m * BC : (m + 1) * BC],
                    func=AF.Identity,
                    bias=sb_bias[:, m : m + 1],
                )
            gxp = bigpool.tile([128, T * 128], BF16, tag="gxp")
            gxp_v = gxp[:].rearrange("p (t m j) -> p m j t", t=T, m=8)
            for m in range(8):
                ps_b = pspool.tile([128, 512], F32, tag="scratch")
                nc.tensor.matmul(
                    out=ps_b[:],
                    lhsT=sb_wih[:, 1, m * 128 : (m + 1) * 128],
                    rhs=wT[:],
                    start=True,
                    stop=True,
                )
                nc.vector.tensor_add(
                    out=gxp_v[:, m],
                    in0=ps_b[:].rearrange("p (t j) -> p j t", t=T),
                    in1=sbias[:, m * BC : (m + 1) * BC].to_broadcast([128, BC, T]),
                )

            # ---- resident linear_w.T (bf16, 125KB/partition), low cols first ----
            lin_sb = cpool.tile([128, 2, V], BF16, tag="lin")
            lin_src = d_lin[:].rearrange("k p n -> p k n")
            dma_engines = [nc.sync, nc.scalar, nc.gpsimd]
            for s in range(8):
                sl = slice(s * (V // 8), (s + 1) * (V // 8))
                dma_engines[s % 3].dma_start(
                    out=lin_sb[:, :, sl], in_=lin_src[:, :, sl]
                )

            # ---- LSTM recurrence + interleaved projection ----
            # hsP[p, k*PP + pos[t] + j] = h_t[k*128+p, j]  (bf16, packed cols)
            hsP = bigpool.tile([128, 2 * PP], BF16, tag="hsP")
            hsP_k = hsP[:].rearrange("p (k m) -> p k m", k=2)
            c_bufs = [
                bigpool.tile([128, 2 * BC], F32, name=f"cbuf{i}", tag=f"cbuf{i}")
                for i in range(2)
            ]

            nmc_full = PP // 128  # full 128-row projection chunks
            mtail = PP - nmc_full * 128  # leftover rows (col-tiled tail)
            tasks = []  # (mc, n)
            emitted_chunks = [False] * nmc_full
            ob_engines = [nc.sync, nc.gpsimd]
            ob_ctr = [0]
            in_drain = [False]

            def emit_proj(ntasks):
                for _ in range(ntasks):
                    if not tasks:
                        return
                    mc, n = tasks.pop(0)
                    nn_ = NV_CHUNK if n < nfull else nrem
                    nsl = slice(n * NV_CHUNK, n * NV_CHUNK + nn_)
                    ps_p = psproj.tile([128, NV_CHUNK], F32, tag="proj")
                    for k in range(2):
                        nc.tensor.matmul(
                            out=ps_p[:, 0:nn_],
                            lhsT=hsP_k[:, k, mc * 128 : (mc + 1) * 128],
                            rhs=lin_sb[:, k, nsl],
                            start=(k == 0),
                            stop=(k == 1),
                        )
                    ob = obpool.tile([128, NV_CHUNK], F32, tag="ob")
                    if in_drain[0] and ob_ctr[0] % 2 == 1:
                        nc.scalar.copy(ob[:, 0:nn_], ps_p[:, 0:nn_])
                    else:
                        nc.vector.tensor_copy(ob[:, 0:nn_], ps_p[:, 0:nn_])
                    eng = ob_engines[ob_ctr[0] % 2]
                    ob_ctr[0] += 1
                    eng.dma_start(
                        out=d_out[mc * 128 : (mc + 1) * 128, nsl],
                        in_=ob[:, 0:nn_],
                    )

            for t in range(T):
                w = cm[t]
                ps_g = psg_pool.tile([128, 128], F32, tag="gate")
                # inject precomputed x-part gates into PSUM via identity matmul
                nc.tensor.matmul(
                    out=ps_g[:],
                    lhsT=ident_b[:],
                    rhs=gxp[:, t * 128 : (t + 1) * 128],
                    start=True,
                    stop=(t == 0),
                    skip_group_check=True,
                )
                if t > 0:
                    for m in range(8):
                        for k in range(2):
                            nc.tensor.matmul(
                                out=ps_g[:, m * BC : m * BC + w],
                                lhsT=sb_whh[:, k, m * 128 : (m + 1) * 128],
                                rhs=hsP_k[:, k, pos[t - 1] : pos[t - 1] + w],
                                start=False,
                                stop=(m == 7 and k == 1),
                                skip_group_check=True,
                            )
                A = wpool.tile([128, 128], F32, tag="At")
                nc.scalar.activation(out=A[:, 0:96], in_=ps_g[:, 0:96], func=AF.Sigmoid)
                nc.scalar.activation(out=A[:, 96:128], in_=ps_g[:, 96:128], func=AF.Tanh)
                t1 = wpool.tile([128, 2 * BC], F32, tag="t1")
                nc.vector.tensor_mul(t1[:], A[:, 0:32], A[:, 96:128])
                c_new = c_bufs[t % 2]
                if t == 0:
                    nc.vector.tensor_copy(c_new[:], t1[:])
                else:
                    c_old = c_bufs[(t - 1) % 2]
                    nc.vector.tensor_mul(c_new[:], A[:, 32:64], c_old[:])
                    nc.vector.tensor_add(c_new[:], c_new[:], t1[:])
                tct = wpool.tile([128, 2 * BC], F32, tag="tct")
                nc.scalar.activation(out=tct[:], in_=c_new[:], func=AF.Tanh)
                # write h packed (only the first cm[t] columns are kept)
                hw = hsP[:].rearrange("p (k m) -> p k m", k=2)[
                    :, :, pos[t] : pos[t] + w
                ]
                nc.vector.tensor_mul(
                    out=hw,
                    in0=A[:, 64:96].rearrange("p (k j) -> p k j", k=2)[:, :, 0:w],
                    in1=tct[:].rearrange("p (k j) -> p k j", k=2)[:, :, 0:w],
                )
                # release projection tasks for chunks fully written
                for mc in range(nmc_full):
                    if not emitted_chunks[mc] and pos[t + 1] >= (mc + 1) * 128:
                        emitted_chunks[mc] = True
                        tasks.extend((mc, n) for n in range(nchunks))
                if t >= 1 and t < T - 1:
                    budget = int(np.ceil(len(tasks) / max(1, T - 1 - t)))
                    emit_proj(budget)
            in_drain[0] = True
            emit_proj(len(tasks) + 1)

            # ---- projection tail ----
            if mtail > 32:
                # plain row-chunk tail (rare: only when >32 leftover rows)
                for n in range(nchunks):
                    nn_ = NV_CHUNK if n < nfull else nrem
                    nsl = slice(n * NV_CHUNK, n * NV_CHUNK + nn_)
                    ps_p = psproj.tile([128, NV_CHUNK], F32, tag="proj")
                    for k in range(2):
                        nc.tensor.matmul(
                            out=ps_p[0:mtail, 0:nn_],
                            lhsT=hsP_k[:, k, nmc_full * 128 : PP],
                            rhs=lin_sb[:, k, nsl],
                            start=(k == 0),
                            stop=(k == 1),
                        )
                    ob = obpool.tile([128, NV_CHUNK], F32, tag="ob")
                    nc.vector.tensor_copy(ob[0:mtail, 0:nn_], ps_p[0:mtail, 0:nn_])
                    ob_engines[n % 2].dma_start(
                        out=d_out[nmc_full * 128 : PP, nsl], in_=ob[0:mtail, 0:nn_]
                    )
            # col-tiled tail: mtail<=32 rows packed into the 4 PE col-groups
            elif mtail > 0:
                for g in range((nchunks + 3) // 4):
                    ns = [n for n in (4 * g + j for j in range(4)) if n < nchunks]
                    ps_p = psproj.tile([128, NV_CHUNK], F32, tag="proj")
                    for j, n in enumerate(ns):
                        nn_ = NV_CHUNK if n < nfull else nrem
                        nsl = slice(n * NV_CHUNK, n * NV_CHUNK + nn_)
                        for k in range(2):
                            nc.tensor.matmul(
                                out=ps_p[32 * j : 32 * j + mtail, 0:nn_],
                                lhsT=hsP_k[:, k, nmc_full * 128 : PP],
                                rhs=lin_sb[:, k, nsl],
                                tile_position=(0, 32 * j),
                                start=(k == 0),
                                stop=(k == 1),
                                skip_group_check=True,
                            )
                    ob = obpool.tile([128, NV_CHUNK], F32, tag="ob")
                    if g % 3 == 2:
                        nc.scalar.copy(ob[:], ps_p[:])
                    else:
                        nc.vector.tensor_copy(ob[:], ps_p[:])
                    for j, n in enumerate(ns):
                        nn_ = NV_CHUNK if n < nfull else nrem
                        nsl = slice(n * NV_CHUNK, n * NV_CHUNK + nn_)
                        ob_engines[j % 2].dma_start(
                            out=d_out[nmc_full * 128 : PP, nsl],
                            in_=ob[32 * j : 32 * j + mtail, 0:nn_],
                        )

            # ---- expert MLP (all 4 domains, one-hot select) ----
            ps_e1 = pspool.tile([128, D * BC], F32, tag="scratch")
            h1 = wpool.tile([128, D * BC], BF16, tag="h1")
            for d in range(D):
                nc.tensor.matmul(
                    out=ps_e1[:, d * BC : (d + 1) * BC],
                    lhsT=sb_w0[:, d, :],
                    rhs=uvT_b[:],
                    start=True,
                    stop=True,
                )
            for d in range(D):
                nc.scalar.activation(
                    out=h1[:, d * BC : (d + 1) * BC],
                    in_=ps_e1[:, d * BC : (d + 1) * BC],
                    func=AF.Relu,
                    bias=sb_b0[:, d : d + 1],
                )
            ps_e2 = pspool.tile([64, D * BC], F32, tag="scratch")
            h2 = wpool.tile([64, D * BC], BF16, tag="h2")
            for d in range(D):
                nc.tensor.matmul(
                    out=ps_e2[:, d * BC : (d + 1) * BC],
                    lhsT=sb_w1[:, d, :],
                    rhs=h1[:, d * BC : (d + 1) * BC],
                    start=True,
                    stop=True,
                )
            for d in range(D):
                nc.scalar.activation(
                    out=h2[:, d * BC : (d + 1) * BC],
                    in_=ps_e2[:, d * BC : (d + 1) * BC],
                    func=AF.Relu,
                    bias=sb_b1[:, d : d + 1],
                )
            ps_e3 = pspool.tile([1, D * BC], F32, tag="scratch")
            for d in range(D):
                nc.tensor.matmul(
                    out=ps_e3[:, d * BC : (d + 1) * BC],
                    lhsT=sb_aw[:],
                    rhs=h2[:, d * BC : (d + 1) * BC],
                    start=True,
                    stop=True,
                )
            r_sig = wpool.tile([1, D * BC], F32, tag="r_sig")
            nc.scalar.activation(out=r_sig[:], in_=ps_e3[:], func=AF.Sigmoid, bias=sb_ab[0:1, 0:1])
            r5 = wpool.tile([1, D * BC], F32, tag="r5")
            nc.vector.tensor_scalar_mul(r5[:], r_sig[:], 5.0)
            nc.vector.tensor_mul(r5[:], r5[:], sb_mask[:])
            r_fin = wpool.tile([1, BC], F32, tag="r_fin")
            nc.vector.tensor_add(r_fin[:], r5[:, 0:BC], r5[:, BC : 2 * BC])
            nc.vector.tensor_add(r_fin[:], r_fin[:], r5[:, 2 * BC : 3 * BC])
            nc.vector.tensor_add(r_fin[:], r_fin[:], r5[:, 3 * BC : 4 * BC])
            nc.sync.dma_start(out=d_rate[:], in_=r_fin[:])

    _split_multiwaits(nc)
    return nc


_NC_CACHE = {}


def _get_nc(cm):
    key = tuple(cm)
    if key not in _NC_CACHE:
        _NC_CACHE[key] = build_program(cm)
    return _NC_CACHE[key]


def _prep_inputs(inputs):
    """Build the 8 per-core in_maps + host-side assembly metadata."""
    u_idx = np.asarray(inputs["u_idx"]).astype(np.int32)
    i_idx = np.asarray(inputs["i_idx"]).astype(np.int32)
    d_idx = np.asarray(inputs["d_idx"]).astype(np.int32)
    review = np.asarray(inputs["review"]).astype(np.int32)
    length = np.asarray(inputs["length"]).astype(np.int32)

    utab = np.ascontiguousarray(np.asarray(inputs["emb_users_w"], np.float32))
    itab = np.ascontiguousarray(
        np.asarray(inputs["emb_items_w"], np.float32).reshape(D * NI, ID)
    )
    wtab = np.ascontiguousarray(np.asarray(inputs["word_emb_w"], np.float32))

    w_ih = np.asarray(inputs["w_ih"], np.float32)[GATE_PERM]
    w_hh = np.asarray(inputs["w_hh"], np.float32)[GATE_PERM]
    bsum = (
        np.asarray(inputs["b_ih"], np.float32) + np.asarray(inputs["b_hh"], np.float32)
    )[GATE_PERM]
    wihT = np.ascontiguousarray(w_ih.T.reshape(2, 128, 4 * H).astype(ml_dtypes.bfloat16))
    whhT = np.ascontiguousarray(w_hh.T.reshape(2, 128, 4 * H).astype(ml_dtypes.bfloat16))
    biasg = np.ascontiguousarray(bsum.reshape(8, 128).T)

    w0 = np.ascontiguousarray(np.asarray(inputs["fc_w0"], np.float32).astype(ml_dtypes.bfloat16))
    b0t = np.ascontiguousarray(np.asarray(inputs["fc_b0"], np.float32).T)
    w1 = np.ascontiguousarray(np.asarray(inputs["fc_w1"], np.float32).astype(ml_dtypes.bfloat16))
    b1t = np.ascontiguousarray(np.asarray(inputs["fc_b1"], np.float32).T)
    affw = np.ascontiguousarray(np.asarray(inputs["affine_w"], np.float32).astype(ml_dtypes.bfloat16))
    affb = np.asarray(inputs["affine_b"], np.float32).reshape(1, 1)

    linear_w = np.asarray(inputs["linear_w"], np.float32)
    linT = np.ascontiguousarray(linear_w.T.reshape(2, 128, V).astype(ml_dtypes.bfloat16))

    # per-core valid-count profile; use max over cores so one program fits all
    counts_c = np.zeros((NCORES, T), np.int64)
    for c in range(NCORES):
        samples = c + NCORES * np.arange(BC)
        counts_c[c] = (length[samples][None, :] > np.arange(T)[:, None]).sum(1)
    cm = counts_c.max(0)

    in_maps = []
    meta = []
    for c in range(NCORES):
        samples = c + NCORES * np.arange(BC)  # descending lengths
        uidx_p = np.zeros((128, 1), np.int32)
        uidx_p[:BC, 0] = u_idx[samples]
        vidx_p = np.zeros((128, 1), np.int32)
        vidx_p[:BC, 0] = d_idx[samples] * NI + i_idx[samples]
        toks_tm = review[samples].T.reshape(T * BC)  # t-major
        ridx = np.ascontiguousarray(toks_tm.reshape(4, 128).T.astype(np.int32))
        dmask = np.zeros((1, D * BC), np.float32)
        for j, b in enumerate(samples):
            dmask[0, d_idx[b] * BC + j] = 1.0
        in_maps.append(
            {
                "uidx": uidx_p,
                "vidx": vidx_p,
                "ridx": ridx,
                "utab": utab,
                "itab": itab,
                "wtab": wtab,
                "wihT": wihT,
                "whhT": whhT,
                "biasg": biasg,
                "w0": w0,
                "b0t": b0t,
                "w1": w1,
                "b1t": b1t,
                "affw": affw,
                "affb": affb,
                "dmask": dmask,
                "linT": linT,
            }
        )
        meta.append((samples, length[samples]))
    return in_maps, meta, length, cm, np.asarray(inputs["linear_b"], np.float32)


def _assemble(results, meta, length, cm, linear_b):
    counts = (length[None, :] > np.arange(T)[:, None]).sum(1)  # global per-t
    cum = np.concatenate([[0], np.cumsum(counts)])
    pos = np.concatenate([[0], np.cumsum(cm)]).astype(int)
    P = int(cum[-1])
    outputs = np.empty((P, V), np.float32)
    rating = np.empty((B, 1), np.float32)
    for c in range(NCORES):
        samples, lens = meta[c]
        res = results[c]
        rating[samples, 0] = res["rating"][0]
        tt, jj = np.meshgrid(np.arange(T), np.arange(BC), indexing="ij")
        valid = tt < lens[jj]
        src = (pos[tt] + jj)[valid]
        dst = (cum[tt] + samples[jj])[valid]
        outputs[dst] = res["outp"][src]
    outputs += linear_b[None, :]
    return rating, outputs


def run(inputs, trace=False):
    in_maps, meta, length, cm, linear_b = _prep_inputs(inputs)
    nc = _get_nc(cm)
    res = run_bass_kernel_spmd(nc, in_maps, core_ids=list(range(NCORES)), trace=trace)
    rating, outputs = _assemble(res.results, meta, length, cm, linear_b)
    return rating, outputs, res


def kernel(**inputs):
    rating, outputs, _ = run(inputs, trace=False)
    return rating, outputs


# revision 16
# speedup vs baseline: 1.6392x; 1.1078x over previous
"""Trainium2 Bass kernel for nn_AutoGenReview_21114059227695 (moe_routing).

Strategy: fully data-parallel over batch B=128 across 8 NeuronCores
(16 samples per core, strided assignment for load balance since lengths
are sorted descending). Each core:
  1. gathers its user/item/word embedding rows on-device (indirect DMA)
  2. runs the domain-routed expert MLP (all 4 experts + one-hot select)
  3. runs the LSTM in feature-major layout (gates.T packed in PSUM; the
     x-part of the gates is bulk-precomputed and re-injected into PSUM
     through an identity matmul so ScalarE reads gates straight from PSUM)
  4. projects its packed hidden states onto the full vocab (row-parallel
     projection, bf16 weights resident in SBUF, fp32 accumulate),
     interleaved into the LSTM step loop as packed columns become ready
Host reassembles the time-major packed output rows and adds linear_b.
"""

import sys

for _p in ("/opt/trn_rl_repo",):
    if _p not in sys.path:
        sys.path.insert(0, _p)

import numpy as np
import ml_dtypes

import concourse.bass as bass
import concourse.tile as tile
from concourse import mybir
from concourse.bass_utils import run_bass_kernel_spmd
from concourse.masks import make_identity
from concourse.vector_clock import ScopedClock, VectorClock

F32 = mybir.dt.float32
BF16 = mybir.dt.bfloat16
I32 = mybir.dt.int32
AF = mybir.ActivationFunctionType

# problem constants
B, T = 128, 32
U, D, NI = 200000, 4, 50000
UD, ID = 64, 64
V, VW, H = 32000, 128, 256
NCORES = 8
BC = B // NCORES  # 16 samples per core
NV_CHUNK = 512  # vocab tile (psum bank limit, fp32)
OUT_DT = mybir.dt.bfloat16  # output staging/store dtype (host upcasts)

# gate permutation: torch order [i f g o] -> kernel order [i f o g] so a
# single ScalarE sigmoid covers cols 0:96 and tanh covers 96:128
GATE_PERM = np.r_[0:256, 256:512, 768:1024, 512:768]


# ---------------------------------------------------------------------------
# Tile drain workaround: walrus on this image rejects >1 sem wait on the
# final TPB_CTRL drain; split the waits across a chain of SP nops.
def _patched_drain_and_barrier(self, tick_clock, wait_clock):
    nc = self.nc
    gc = tick_clock.global_clock
    nprocs = len(gc)
    for i in range(nprocs):
        t = gc[i]
        if t > 0:
            nop_inst = nc.sync.nop(nofuse=True, hint=f"drain_split_{i}")
            vc = VectorClock([0] * nprocs)
            vc.require_at_least(i, t)
            wait_clock.add_sem_waits(nop_inst.ins, ScopedClock({None: vc}))
    nc.sync.drain()
    nc.all_engine_barrier()
    assert self.sems is not None
    popped = nc._tile_sem_poison_stack.pop()
    assert popped is self._sem_poison
    nc.clear_and_free_semaphores(list(self.sems.allocated().values()))
    nc.all_engine_barrier()


tile.TileContext._drain_and_barrier = _patched_drain_and_barrier


def _split_multiwaits(nc):
    """walrus on this image encodes at most one sem-wait per instruction;
    hoist extra waits onto single-wait NoOps immediately preceding the
    instruction on the same engine (in-order sequencers make this
    semantically identical)."""
    fn = nc.m.functions[0]
    ctr = 0
    for b in fn.blocks:
        out = []
        changed = False
        for inst in b.instructions:
            si = inst.sync_info
            if si is not None and len(si.on_wait) > 1:
                changed = True
                waits = list(si.on_wait)
                for w in waits[:-1]:
                    ctr += 1
                    out.append(
                        mybir.InstNoOp(
                            name=f"WSPLIT-{ctr}",
                            engine=inst.engine,
                            sync_info=mybir.SyncInfo(on_wait=[w], on_update=[]),
                        )
                    )
                inst.sync_info = mybir.SyncInfo(
                    on_wait=[waits[-1]], on_update=list(si.on_update)
                )
            out.append(inst)
        if changed:
            b.instructions = out


# ---------------------------------------------------------------------------
def build_program(cm):
    """cm: per-timestep packed column width (max over cores of per-core
    valid-sample counts), non-increasing, cm[0] == BC."""
    cm = list(cm)
    pos = np.concatenate([[0], np.cumsum(cm)]).astype(int)  # packed offsets
    PP = int(pos[-1])  # packed columns per core
    nmc = (PP + 127) // 128  # projection row chunks

    nc = bass.Bass()

    d_uidx = nc.declare_dram_parameter("uidx", [128, 1], I32, isOutput=False)
    d_vidx = nc.declare_dram_parameter("vidx", [128, 1], I32, isOutput=False)
    d_ridx = nc.declare_dram_parameter("ridx", [128, 4], I32, isOutput=False)
    d_utab = nc.declare_dram_parameter("utab", [U, UD], F32, isOutput=False)
    d_itab = nc.declare_dram_parameter("itab", [D * NI, ID], F32, isOutput=False)
    d_wtab = nc.declare_dram_parameter("wtab", [V, VW], F32, isOutput=False)
    d_wih = nc.declare_dram_parameter("wihT", [2, 128, 4 * H], BF16, isOutput=False)
    d_whh = nc.declare_dram_parameter("whhT", [2, 128, 4 * H], BF16, isOutput=False)
    d_bias = nc.declare_dram_parameter("biasg", [128, 8], F32, isOutput=False)
    d_w0 = nc.declare_dram_parameter("w0", [D, 128, 128], BF16, isOutput=False)
    d_b0 = nc.declare_dram_parameter("b0t", [128, D], F32, isOutput=False)
    d_w1 = nc.declare_dram_parameter("w1", [D, 128, 64], BF16, isOutput=False)
    d_b1 = nc.declare_dram_parameter("b1t", [64, D], F32, isOutput=False)
    d_aw = nc.declare_dram_parameter("affw", [64, 1], BF16, isOutput=False)
    d_ab = nc.declare_dram_parameter("affb", [1, 1], F32, isOutput=False)
    d_mask = nc.declare_dram_parameter("dmask", [1, D * BC], F32, isOutput=False)
    d_lin = nc.declare_dram_parameter("linT", [2, 128, V], BF16, isOutput=False)
    d_out = nc.declare_dram_parameter("outp", [PP, V], OUT_DT, isOutput=True)
    d_rate = nc.declare_dram_parameter("rating", [1, BC], F32, isOutput=True)

    nfull, nrem = divmod(V, NV_CHUNK)
    nchunks = nfull + (1 if nrem else 0)

    with tile.TileContext(nc) as tc:
        with (
            tc.tile_pool(name="const", bufs=1) as cpool,
            tc.tile_pool(name="work", bufs=2) as wpool,
            tc.tile_pool(name="big", bufs=1) as bigpool,
            tc.tile_pool(name="ob", bufs=4) as obpool,
            tc.tile_pool(name="ps", bufs=2, space="PSUM") as pspool,
            tc.tile_pool(name="psproj", bufs=4, space="PSUM") as psproj,
            tc.tile_pool(name="psg", bufs=2, space="PSUM") as psg_pool,
        ):
            # ---- small loads first: indices, LSTM/expert weights ----
            sb_uidx = cpool.tile([128, 1], I32, tag="uidx")
            nc.sync.dma_start(out=sb_uidx[:], in_=d_uidx[:])
            sb_vidx = cpool.tile([128, 1], I32, tag="vidx")
            nc.sync.dma_start(out=sb_vidx[:], in_=d_vidx[:])
            sb_ridx = cpool.tile([128, 4], I32, tag="ridx")
            nc.sync.dma_start(out=sb_ridx[:], in_=d_ridx[:])

            sb_wih = cpool.tile([128, 2, 4 * H], BF16, tag="wih")
            nc.sync.dma_start(out=sb_wih[:], in_=d_wih[:].rearrange("k p n -> p k n"))
            sb_whh = cpool.tile([128, 2, 4 * H], BF16, tag="whh")
            nc.sync.dma_start(out=sb_whh[:], in_=d_whh[:].rearrange("k p n -> p k n"))
            sb_bias = cpool.tile([128, 8], F32, tag="biasg")
            nc.sync.dma_start(out=sb_bias[:], in_=d_bias[:])

            sb_w0 = cpool.tile([128, D, 128], BF16, tag="w0")
            nc.scalar.dma_start(out=sb_w0[:], in_=d_w0[:].rearrange("d k m -> k d m"))
            sb_b0 = cpool.tile([128, D], F32, tag="b0")
            nc.scalar.dma_start(out=sb_b0[:], in_=d_b0[:])
            sb_w1 = cpool.tile([128, D, 64], BF16, tag="w1")
            nc.scalar.dma_start(out=sb_w1[:], in_=d_w1[:].rearrange("d k m -> k d m"))
            sb_b1 = cpool.tile([64, D], F32, tag="b1")
            nc.scalar.dma_start(out=sb_b1[:], in_=d_b1[:])
            sb_aw = cpool.tile([64, 1], BF16, tag="affw")
            nc.scalar.dma_start(out=sb_aw[:], in_=d_aw[:])
            sb_ab = cpool.tile([1, 1], F32, tag="affb")
            nc.scalar.dma_start(out=sb_ab[:], in_=d_ab[:])
            sb_mask = cpool.tile([1, D * BC], F32, tag="dmask")
            nc.scalar.dma_start(out=sb_mask[:], in_=d_mask[:])

            ident = cpool.tile([128, 128], F32, tag="ident")
            make_identity(nc, ident[:])
            ident_b = cpool.tile([128, 128], BF16, tag="identb")
            nc.vector.tensor_copy(ident_b[:], ident[:])

            # ---- embedding gathers (indirect DMA: one row per partition) ----
            uv_sb = wpool.tile([128, 128], F32, tag="uvrows")
            nc.gpsimd.indirect_dma_start(
                out=uv_sb[:, 0:UD],
                out_offset=None,
                in_=d_utab[:],
                in_offset=bass.IndirectOffsetOnAxis(ap=sb_uidx[:, :1], axis=0),
            )
            nc.gpsimd.indirect_dma_start(
                out=uv_sb[:, UD : UD + ID],
                out_offset=None,
                in_=d_itab[:],
                in_offset=bass.IndirectOffsetOnAxis(ap=sb_vidx[:, :1], axis=0),
            )

            ps_t = pspool.tile([128, 128], F32, tag="scratch")
            nc.tensor.transpose(out=ps_t[:], in_=uv_sb[:], identity=ident[:])
            uvT_b = wpool.tile([128, BC], BF16, tag="uvT_b")
            nc.vector.tensor_copy(uvT_b[:], ps_t[:, 0:BC])

            wT = bigpool.tile([128, T * BC], BF16, tag="wT")
            for g in range(4):
                wg = wpool.tile([128, 128], F32, tag="wg")
                nc.gpsimd.indirect_dma_start(
                    out=wg[:],
                    out_offset=None,
                    in_=d_wtab[:],
                    in_offset=bass.IndirectOffsetOnAxis(ap=sb_ridx[:, g : g + 1], axis=0),
                )
                ps_w = pspool.tile([128, 128], F32, tag="scratch")
                nc.tensor.transpose(out=ps_w[:], in_=wg[:], identity=ident[:])
                nc.vector.tensor_copy(wT[:, g * 128 : (g + 1) * 128], ps_w[:])

            # ---- LSTM bulk x-part (bf16 gxp, packed gate layout) ----
            ps_guv = psg_pool.tile([128, 128], F32, tag="gate")
            for m in range(8):
                nc.tensor.matmul(
                    out=ps_guv[:, m * BC : (m + 1) * BC],
                    lhsT=sb_wih[:, 0, m * 128 : (m + 1) * 128],
                    rhs=uvT_b[:],
                    start=True,
                    stop=True,
                )
            sbias = wpool.tile([128, 128], F32, tag="sbias")
            for m in range(8):
                nc.scalar.activation(
                    out=sbias[:, m * BC : (m + 1) * BC],
                    in_=ps_guv[:, m * BC : (m + 1) * BC],
                    func=AF.Identity,
                    bias=sb_bias[:, m : m + 1],
                )
            gxp = bigpool.tile([128, T * 128], BF16, tag="gxp")
            gxp_v = gxp[:].rearrange("p (t m j) -> p m j t", t=T, m=8)
            for m in range(8):
                ps_b = pspool.tile([128, 512], F32, tag="scratch")
                nc.tensor.matmul(
                    out=ps_b[:],
                    lhsT=sb_wih[:, 1, m * 128 : (m + 1) * 128],
                    rhs=wT[:],
                    start=True,
                    stop=True,
                )
                nc.vector.tensor_add(
                    out=gxp_v[:, m],
                    in0=ps_b[:].rearrange("p (t j) -> p j t", t=T),
                    in1=sbias[:, m * BC : (m + 1) * BC].to_broadcast([128, BC, T]),
                )

            # ---- resident linear_w.T (bf16, 125KB/partition), low cols first ----
            lin_sb = cpool.tile([128, 2, V], BF16, tag="lin")
            lin_src = d_lin[:].rearrange("k p n -> p k n")
            dma_engines = [nc.sync, nc.scalar, nc.gpsimd]
            for s in range(8):
                sl = slice(s * (V // 8), (s + 1) * (V // 8))
                dma_engines[s % 3].dma_start(
                    out=lin_sb[:, :, sl], in_=lin_src[:, :, sl]
                )

            # ---- LSTM recurrence + interleaved projection ----
            # hsP[p, k*PP + pos[t] + j] = h_t[k*128+p, j]  (bf16, packed cols)
            hsP = bigpool.tile([128, 2 * PP], BF16, tag="hsP")
            hsP_k = hsP[:].rearrange("p (k m) -> p k m", k=2)
            c_bufs = [
                bigpool.tile([128, 2 * BC], F32, name=f"cbuf{i}", tag=f"cbuf{i}")
                for i in range(2)
            ]

            nmc_full = PP // 128  # full 128-row projection chunks
            mtail = PP - nmc_full * 128  # leftover rows (col-tiled tail)
            tasks = []  # (mc, n)
            emitted_chunks = [False] * nmc_full
            ob_engines = [nc.sync, nc.gpsimd]
            ob_ctr = [0]
            in_drain = [False]

            def emit_proj(ntasks):
                for _ in range(ntasks):
                    if not tasks:
                        return
                    mc, n = tasks.pop(0)
                    nn_ = NV_CHUNK if n < nfull else nrem
                    nsl = slice(n * NV_CHUNK, n * NV_CHUNK + nn_)
                    ps_p = psproj.tile([128, NV_CHUNK], F32, tag="proj")
                    for k in range(2):
                        nc.tensor.matmul(
                            out=ps_p[:, 0:nn_],
                            lhsT=hsP_k[:, k, mc * 128 : (mc + 1) * 128],
                            rhs=lin_sb[:, k, nsl],
                            start=(k == 0),
                            stop=(k == 1),
                        )
                    ob = obpool.tile([128, NV_CHUNK], OUT_DT, tag="ob")
                    if in_drain[0] and ob_ctr[0] % 2 == 1:
                        nc.scalar.copy(ob[:, 0:nn_], ps_p[:, 0:nn_])
                    else:
                        nc.vector.tensor_copy(ob[:, 0:nn_], ps_p[:, 0:nn_])
                    eng = ob_engines[ob_ctr[0] % 2]
                    ob_ctr[0] += 1
                    eng.dma_start(
                        out=d_out[mc * 128 : (mc + 1) * 128, nsl],
                        in_=ob[:, 0:nn_],
                    )

            for t in range(T):
                w = cm[t]
                ps_g = psg_pool.tile([128, 128], F32, tag="gate")
                # inject precomputed x-part gates into PSUM via identity matmul
                nc.tensor.matmul(
                    out=ps_g[:],
                    lhsT=ident_b[:],
                    rhs=gxp[:, t * 128 : (t + 1) * 128],
                    start=True,
                    stop=(t == 0),
                    skip_group_check=True,
                )
                if t > 0:
                    for m in range(8):
                        for k in range(2):
                            nc.tensor.matmul(
                                out=ps_g[:, m * BC : m * BC + w],
                                lhsT=sb_whh[:, k, m * 128 : (m + 1) * 128],
                                rhs=hsP_k[:, k, pos[t - 1] : pos[t - 1] + w],
                                start=False,
                                stop=(m == 7 and k == 1),
                                skip_group_check=True,
                            )
                A = wpool.tile([128, 128], F32, tag="At")
                nc.scalar.activation(out=A[:, 0:96], in_=ps_g[:, 0:96], func=AF.Sigmoid)
                nc.scalar.activation(out=A[:, 96:128], in_=ps_g[:, 96:128], func=AF.Tanh)
                t1 = wpool.tile([128, 2 * BC], F32, tag="t1")
                nc.vector.tensor_mul(t1[:], A[:, 0:32], A[:, 96:128])
                c_new = c_bufs[t % 2]
                if t == 0:
                    nc.vector.tensor_copy(c_new[:], t1[:])
                else:
                    c_old = c_bufs[(t - 1) % 2]
                    nc.vector.tensor_mul(c_new[:], A[:, 32:64], c_old[:])
                    nc.vector.tensor_add(c_new[:], c_new[:], t1[:])
                tct = wpool.tile([128, 2 * BC], F32, tag="tct")
                nc.scalar.activation(out=tct[:], in_=c_new[:], func=AF.Tanh)
                # write h packed (only the first cm[t] columns are kept)
                hw = hsP[:].rearrange("p (k m) -> p k m", k=2)[
                    :, :, pos[t] : pos[t] + w
                ]
                nc.vector.tensor_mul(
                    out=hw,
                    in0=A[:, 64:96].rearrange("p (k j) -> p k j", k=2)[:, :, 0:w],
                    in1=tct[:].rearrange("p (k j) -> p k j", k=2)[:, :, 0:w],
                )
                # release projection tasks for chunks fully written
                for mc in range(nmc_full):
                    if not emitted_chunks[mc] and pos[t + 1] >= (mc + 1) * 128:
                        emitted_chunks[mc] = True
                        tasks.extend((mc, n) for n in range(nchunks))
                if t >= 1 and t < T - 1:
                    budget = int(np.ceil(len(tasks) / max(1, T - 1 - t)))
                    emit_proj(budget)
            in_drain[0] = True
            emit_proj(len(tasks) + 1)

            # ---- projection tail ----
            if mtail > 32:
                # plain row-chunk tail (rare: only when >32 leftover rows)
                for n in range(nchunks):
                    nn_ = NV_CHUNK if n < nfull else nrem
                    nsl = slice(n * NV_CHUNK, n * NV_CHUNK + nn_)
                    ps_p = psproj.tile([128, NV_CHUNK], F32, tag="proj")
                    for k in range(2):
                        nc.tensor.matmul(
                            out=ps_p[0:mtail, 0:nn_],
                            lhsT=hsP_k[:, k, nmc_full * 128 : PP],
                            rhs=lin_sb[:, k, nsl],
                            start=(k == 0),
                            stop=(k == 1),
                        )
                    ob = obpool.tile([128, NV_CHUNK], OUT_DT, tag="ob")
                    nc.vector.tensor_copy(ob[0:mtail, 0:nn_], ps_p[0:mtail, 0:nn_])
                    ob_engines[n % 2].dma_start(
                        out=d_out[nmc_full * 128 : PP, nsl], in_=ob[0:mtail, 0:nn_]
                    )
            # col-tiled tail: mtail<=32 rows packed into the 4 PE col-groups
            elif mtail > 0:
                for g in range((nchunks + 3) // 4):
                    ns = [n for n in (4 * g + j for j in range(4)) if n < nchunks]
                    ps_p = psproj.tile([128, NV_CHUNK], F32, tag="proj")
                    for j, n in enumerate(ns):
                        nn_ = NV_CHUNK if n < nfull else nrem
                        nsl = slice(n * NV_CHUNK, n * NV_CHUNK + nn_)
                        for k in range(2):
                            nc.tensor.matmul(
                                out=ps_p[32 * j : 32 * j + mtail, 0:nn_],
                                lhsT=hsP_k[:, k, nmc_full * 128 : PP],
                                rhs=lin_sb[:, k, nsl],
                                tile_position=(0, 32 * j),
                                start=(k == 0),
                                stop=(k == 1),
                                skip_group_check=True,
                            )
                    ob = obpool.tile([128, NV_CHUNK], OUT_DT, tag="ob")
                    if g % 3 == 2:
                        nc.scalar.copy(ob[:], ps_p[:])
                    else:
                        nc.vector.tensor_copy(ob[:], ps_p[:])
                    for j, n in enumerate(ns):
                        nn_ = NV_CHUNK if n < nfull else nrem
                        nsl = slice(n * NV_CHUNK, n * NV_CHUNK + nn_)
                        ob_engines[j % 2].dma_start(
                            out=d_out[nmc_full * 128 : PP, nsl],
                            in_=ob[32 * j : 32 * j + mtail, 0:nn_],
                        )

            # ---- expert MLP (all 4 domains, one-hot select) ----
            ps_e1 = pspool.tile([128, D * BC], F32, tag="scratch")
            h1 = wpool.tile([128, D * BC], BF16, tag="h1")
            for d in range(D):
                nc.tensor.matmul(
                    out=ps_e1[:, d * BC : (d + 1) * BC],
                    lhsT=sb_w0[:, d, :],
                    rhs=uvT_b[:],
                    start=True,
                    stop=True,
                )
            for d in range(D):
                nc.scalar.activation(
                    out=h1[:, d * BC : (d + 1) * BC],
                    in_=ps_e1[:, d * BC : (d + 1) * BC],
                    func=AF.Relu,
                    bias=sb_b0[:, d : d + 1],
                )
            ps_e2 = pspool.tile([64, D * BC], F32, tag="scratch")
            h2 = wpool.tile([64, D * BC], BF16, tag="h2")
            for d in range(D):
                nc.tensor.matmul(
                    out=ps_e2[:, d * BC : (d + 1) * BC],
                    lhsT=sb_w1[:, d, :],
                    rhs=h1[:, d * BC : (d + 1) * BC],
                    start=True,
                    stop=True,
                )
            for d in range(D):
                nc.scalar.activation(
                    out=h2[:, d * BC : (d + 1) * BC],
                    in_=ps_e2[:, d * BC : (d + 1) * BC],
                    func=AF.Relu,
                    bias=sb_b1[:, d : d + 1],
                )
            ps_e3 = pspool.tile([1, D * BC], F32, tag="scratch")
            for d in range(D):
                nc.tensor.matmul(
                    out=ps_e3[:, d * BC : (d + 1) * BC],
                    lhsT=sb_aw[:],
                    rhs=h2[:, d * BC : (d + 1) * BC],
                    start=True,
                    stop=True,
                )
            r_sig = wpool.tile([1, D * BC], F32, tag="r_sig")
            nc.scalar.activation(out=r_sig[:], in_=ps_e3[:], func=AF.Sigmoid, bias=sb_ab[0:1, 0:1])
            r5 = wpool.tile([1, D * BC], F32, tag="r5")
            nc.vector.tensor_scalar_mul(r5[:], r_sig[:], 5.0)
            nc.vector.tensor_mul(r5[:], r5[:], sb_mask[:])
            r_fin = wpool.tile([1, BC], F32, tag="r_fin")
            nc.vector.tensor_add(r_fin[:], r5[:, 0:BC], r5[:, BC : 2 * BC])
            nc.vector.tensor_add(r_fin[:], r_fin[:], r5[:, 2 * BC : 3 * BC])
            nc.vector.tensor_add(r_fin[:], r_fin[:], r5[:, 3 * BC : 4 * BC])
            nc.sync.dma_start(out=d_rate[:], in_=r_fin[:])

    _split_multiwaits(nc)
    return nc


_NC_CACHE = {}


def _get_nc(cm):
    key = tuple(cm)
    if key not in _NC_CACHE:
        _NC_CACHE[key] = build_program(cm)
    return _NC_CACHE[key]


def _prep_inputs(inputs):
    """Build the 8 per-core in_maps + host-side assembly metadata."""
    u_idx = np.asarray(inputs["u_idx"]).astype(np.int32)
    i_idx = np.asarray(inputs["i_idx"]).astype(np.int32)
    d_idx = np.asarray(inputs["d_idx"]).astype(np.int32)
    review = np.asarray(inputs["review"]).astype(np.int32)
    length = np.asarray(inputs["length"]).astype(np.int32)

    utab = np.ascontiguousarray(np.asarray(inputs["emb_users_w"], np.float32))
    itab = np.ascontiguousarray(
        np.asarray(inputs["emb_items_w"], np.float32).reshape(D * NI, ID)
    )
    wtab = np.ascontiguousarray(np.asarray(inputs["word_emb_w"], np.float32))

    w_ih = np.asarray(inputs["w_ih"], np.float32)[GATE_PERM]
    w_hh = np.asarray(inputs["w_hh"], np.float32)[GATE_PERM]
    bsum = (
        np.asarray(inputs["b_ih"], np.float32) + np.asarray(inputs["b_hh"], np.float32)
    )[GATE_PERM]
    wihT = np.ascontiguousarray(w_ih.T.reshape(2, 128, 4 * H).astype(ml_dtypes.bfloat16))
    whhT = np.ascontiguousarray(w_hh.T.reshape(2, 128, 4 * H).astype(ml_dtypes.bfloat16))
    biasg = np.ascontiguousarray(bsum.reshape(8, 128).T)

    w0 = np.ascontiguousarray(np.asarray(inputs["fc_w0"], np.float32).astype(ml_dtypes.bfloat16))
    b0t = np.ascontiguousarray(np.asarray(inputs["fc_b0"], np.float32).T)
    w1 = np.ascontiguousarray(np.asarray(inputs["fc_w1"], np.float32).astype(ml_dtypes.bfloat16))
    b1t = np.ascontiguousarray(np.asarray(inputs["fc_b1"], np.float32).T)
    affw = np.ascontiguousarray(np.asarray(inputs["affine_w"], np.float32).astype(ml_dtypes.bfloat16))
    affb = np.asarray(inputs["affine_b"], np.float32).reshape(1, 1)

    linear_w = np.asarray(inputs["linear_w"], np.float32)
    linT = np.ascontiguousarray(linear_w.T.reshape(2, 128, V).astype(ml_dtypes.bfloat16))

    # per-core valid-count profile; use max over cores so one program fits all
    counts_c = np.zeros((NCORES, T), np.int64)
    for c in range(NCORES):
        samples = c + NCORES * np.arange(BC)
        counts_c[c] = (length[samples][None, :] > np.arange(T)[:, None]).sum(1)
    cm = counts_c.max(0)

    in_maps = []
    meta = []
    for c in range(NCORES):
        samples = c + NCORES * np.arange(BC)  # descending lengths
        uidx_p = np.zeros((128, 1), np.int32)
        uidx_p[:BC, 0] = u_idx[samples]
        vidx_p = np.zeros((128, 1), np.int32)
        vidx_p[:BC, 0] = d_idx[samples] * NI + i_idx[samples]
        toks_tm = review[samples].T.reshape(T * BC)  # t-major
        ridx = np.ascontiguousarray(toks_tm.reshape(4, 128).T.astype(np.int32))
        dmask = np.zeros((1, D * BC), np.float32)
        for j, b in enumerate(samples):
            dmask[0, d_idx[b] * BC + j] = 1.0
        in_maps.append(
            {
                "uidx": uidx_p,
                "vidx": vidx_p,
                "ridx": ridx,
                "utab": utab,
                "itab": itab,
                "wtab": wtab,
                "wihT": wihT,
                "whhT": whhT,
                "biasg": biasg,
                "w0": w0,
                "b0t": b0t,
                "w1": w1,
                "b1t": b1t,
                "affw": affw,
                "affb": affb,
                "dmask": dmask,
                "linT": linT,
            }
        )
        meta.append((samples, length[samples]))
    return in_maps, meta, length, cm, np.asarray(inputs["linear_b"], np.float32)


def _assemble(results, meta, length, cm, linear_b):
    counts = (length[None, :] > np.arange(T)[:, None]).sum(1)  # global per-t
    cum = np.concatenate([[0], np.cumsum(counts)])
    pos = np.concatenate([[0], np.cumsum(cm)]).astype(int)
    P = int(cum[-1])
    outputs = np.empty((P, V), np.float32)
    rating = np.empty((B, 1), np.float32)
    for c in range(NCORES):
        samples, lens = meta[c]
        res = results[c]
        rating[samples, 0] = res["rating"][0]
        tt, jj = np.meshgrid(np.arange(T), np.arange(BC), indexing="ij")
        valid = tt < lens[jj]
        src = (pos[tt] + jj)[valid]
        dst = (cum[tt] + samples[jj])[valid]
        outputs[dst] = res["outp"][src].astype(np.float32)
    outputs += linear_b[None, :]
    return rating, outputs


def run(inputs, trace=False):
    in_maps, meta, length, cm, linear_b = _prep_inputs(inputs)
    nc = _get_nc(cm)
    res = run_bass_kernel_spmd(nc, in_maps, core_ids=list(range(NCORES)), trace=trace)
    rating, outputs = _assemble(res.results, meta, length, cm, linear_b)
    return rating, outputs, res


def kernel(**inputs):
    rating, outputs, _ = run(inputs, trace=False)
    return rating, outputs
